# revision 1
# baseline (speedup 1.0000x reference)
"""Trainium2 Bass kernel for nn_AttentionForONNX (Transformer-XL style
relative-position attention).

Strategy (data-parallel over batch, 2 batches per core on 8 cores):
  - Host pre-transposes x -> xT [B,E,T] (f32) and pos_emb -> peT
    [B,E,S2+1 padded] in bf16; weights transposed, Wp in bf16.
  - Projections: qT/kT = W @ x.T (+bias), v natural = x @ Wv.T + bv packed
    bf16 into a 65-col-per-head layout with a ones column (so the softmax
    denominator falls out of the attn@v matmul), pT = Wp @ pos_emb.T (bf16).
  - Scores computed TRANSPOSED ([s,t] tiles): bd band u [t,640] per t-tile,
    rel_shift via diagonal SBUF->SBUF DMA (2 heads per DMA), then
    identity-moving matmuls transpose the shifted band straight into the
    score PSUM tile where the ac matmul has already accumulated
    (k-stationary x qu-moving gives [s,t] directly).
  - softmax without max-subtraction: ACT exp -> bf16 [s,t] tiles used
    directly as attn@v stationary (no PE transpose of exp, no copyback).
  - attn@v: per head [t,65] PSUM accumulation over s-chunks; col 64 is the
    denominator; DVE reciprocal + per-head scale epilogue -> out.
  - Projection matmuls f32r (1 cycle/row); all bd-path matmuls bf16 moving
    so no small-N f32r penalty.
"""
import sys
import os

for _p in ("/opt/trn_rl_repo", "/root/.axon_site/_ro/trn_rl_repo"):
    if os.path.isdir(_p) and _p not in sys.path:
        sys.path.insert(0, _p)

import numpy as np

B, T, E, H = 16, 512, 512, 8
HD = E // H
S2 = 2 * T - 1
N_CORES = 8
BPC = B // N_CORES          # batches per core
SCALE = 1.0 / float(np.sqrt(HD))

_CACHE = {}


def _split_multiwaits(nc, mybir):
    """walrus supports only one sync-wait per instruction: split extras
    into single-wait NOPs preceding the instruction."""
    n = 0
    for bb in nc.main_func.blocks:
        new_insts = []
        for ins in bb.instructions:
            si = ins.sync_info
            if si and si.on_wait and len(si.on_wait) > 1:
                waits = list(si.on_wait)
                for w in waits[:-1]:
                    nop = mybir.InstNoOp(name=f"{ins.name}-w{n}", ins=[], outs=[])
                    nop.engine = ins.engine
                    nop.sync_info = mybir.SyncInfo(on_wait=[w], on_update=[])
                    nc.register_instruction(nop, overwrite=True)
                    new_insts.append(nop)
                    n += 1
                ins.sync_info = mybir.SyncInfo(on_wait=[waits[-1]],
                                               on_update=list(si.on_update))
            new_insts.append(ins)
        bb.instructions[:] = new_insts
    return n


def _build_nc(bpc=BPC):
    import concourse.bass as bass
    import concourse.mybir as mybir
    import concourse.tile as tile
    from concourse.ap import AP
    from concourse.masks import make_identity

    F32 = mybir.dt.float32
    F32R = mybir.dt.float32r
    BF16 = mybir.dt.bfloat16
    AT = mybir.AluOpType
    AF = mybir.ActivationFunctionType

    nc = bass.Bass("TRN2", target_bir_lowering=False)

    xT = nc.dram_tensor("xT", [bpc, E, T], F32R, kind="ExternalInput")
    peT = nc.dram_tensor("peT", [bpc, E, S2 + 1], BF16, kind="ExternalInput")
    wqT = nc.dram_tensor("wqT", [E, E], F32R, kind="ExternalInput")
    wkT = nc.dram_tensor("wkT", [E, E], F32R, kind="ExternalInput")
    wvT = nc.dram_tensor("wvT", [E, E], F32R, kind="ExternalInput")
    wpT = nc.dram_tensor("wpT", [E, E], BF16, kind="ExternalInput")
    # bias_u / (bias_v - bias_u) / bk packed [128, 4]: col eo = bias[eo*128:+128]
    bu = nc.dram_tensor("bu", [128, 4], F32, kind="ExternalInput")
    bdv = nc.dram_tensor("bdv", [128, 4], F32, kind="ExternalInput")
    bkk = nc.dram_tensor("bkk", [128, 4], F32, kind="ExternalInput")
    bvec = nc.dram_tensor("bvec", [E], F32, kind="ExternalInput")   # bv for v
    out_d = nc.dram_tensor("out", [bpc, T, E], F32, kind="ExternalOutput")

    NEO = E // 128   # 4 tiles along e_out / e_in / s / t
    J0 = [384 - 128 * tt for tt in range(4)]
    W65 = 65 * H     # 520: VSB2 row width

    with tile.TileContext(nc) as tc:
        with (
            tc.tile_pool(name="const", bufs=1) as const,
            tc.tile_pool(name="batch", bufs=1) as batch,
            tc.tile_pool(name="blate", bufs=2) as blate,
            tc.tile_pool(name="ubp", bufs=2) as ubp,
            tc.tile_pool(name="vpp", bufs=2) as vpp,
            tc.tile_pool(name="epp", bufs=2) as epp,
            tc.tile_pool(name="osb", bufs=2) as osb,
            tc.tile_pool(name="work", bufs=2) as work,
            tc.tile_pool(name="pps", bufs=2, space="PSUM") as pps,     # 4 banks
            tc.tile_pool(name="pacs", bufs=2, space="PSUM") as pacs,   # 2 banks
            tc.tile_pool(name="avs", bufs=2, space="PSUM") as avs,     # 2 banks
        ):
            # ---- first-needed inputs first: the first q-projection matmul
            # (eo=0, ei=0) needs only XT0[0] + WQ[0]
            # ---- warmup order: the p projection needs only PET0+WP
            # (12KB/partition, all bf16) so the PE can start ~4.5us in while
            # the fp32 x/weights stream behind it.
            PET0 = batch.tile([128, 4 * (S2 + 1)], BF16, tag="pe0", name="peT0")
            nc.sync.dma_start(
                out=PET0,
                in_=AP(peT, 0, [[1024, 128], [131072, 4], [1, 1024]]))
            WP = const.tile([128, 4 * E], BF16, tag="wp")
            nc.sync.dma_start(out=WP,
                              in_=AP(wpT, 0, [[512, 128], [65536, 4], [1, 512]]))
            XT0 = [batch.tile([128, T], F32R, tag=f"xt0_{ei}", name=f"xT0{ei}")
                   for ei in range(NEO)]
            WQ = [const.tile([128, E], F32R, tag=f"wq{ei}", name=f"wq{ei}")
                  for ei in range(NEO)]
            for ei in range(NEO):
                nc.sync.dma_start(out=XT0[ei],
                                  in_=xT[0, ei * 128:(ei + 1) * 128, :])
                nc.sync.dma_start(out=WQ[ei],
                                  in_=wqT[ei * 128:(ei + 1) * 128, :])
            # small epilogue constants
            BU = const.tile([128, 4], F32, tag="bu")
            nc.sync.dma_start(out=BU, in_=bu[:])
            BDV = const.tile([128, 4], F32, tag="bdv")
            nc.sync.dma_start(out=BDV, in_=bdv[:])
            BK = const.tile([128, 4], F32, tag="bkk")
            nc.sync.dma_start(out=BK, in_=bkk[:])
            BVB = const.tile([128, E], F32, tag="bvb")
            nc.sync.dma_start(out=BVB, in_=AP(bvec, 0, [[0, 128], [1, E]]))
            IDENT = const.tile([128, 128], BF16, tag="ident")
            make_identity(nc, IDENT[:])
            # remaining weights: one batched DMA per tensor, cols 512*ei+c
            WK = const.tile([128, 4 * E], F32R, tag="wk")
            nc.sync.dma_start(out=WK,
                              in_=AP(wkT, 0, [[512, 128], [65536, 4], [1, 512]]))
            WV = const.tile([128, 4 * E], F32R, tag="wv")
            nc.sync.dma_start(out=WV,
                              in_=AP(wvT, 0, [[512, 128], [65536, 4], [1, 512]]))

            def emit_loads(b):
                XTb = batch.tile([128, 4 * T], F32R, tag=f"xt{b}", name=f"xT{b}")
                nc.sync.dma_start(
                    out=XTb,
                    in_=AP(xT, b * E * T, [[512, 128], [65536, 4], [1, 512]]))
                PETb = batch.tile([128, 4 * (S2 + 1)], BF16, tag=f"pe{b}",
                                  name=f"peT{b}")
                nc.sync.dma_start(
                    out=PETb,
                    in_=AP(peT, b * E * (S2 + 1),
                           [[1024, 128], [131072, 4], [1, 1024]]))
                xs = [XTb[:, ei * T:(ei + 1) * T] for ei in range(NEO)]
                ps = [PETb[:, ei * (S2 + 1):(ei + 1) * (S2 + 1)]
                      for ei in range(NEO)]
                return xs, ps

            def emit_proj_qk(b, XTs):
                """q (ACT epilogues -> QU f32r, Pool -> QV bf16) and k."""
                QU, QV, KT = [], [], []
                for eo in range(NEO):
                    pq = pps.tile([128, S2 + 1], F32, tag="pp", name="pq")
                    for ei in range(NEO):
                        nc.tensor.matmul(
                            pq[:, 0:T], WQ[ei][:, eo * 128:(eo + 1) * 128],
                            XTs[ei], start=(ei == 0), stop=(ei == NEO - 1))
                    qu = blate.tile([128, T], F32R, tag=f"qu{eo}", name=f"qu{eo}")
                    nc.vector.tensor_scalar_add(qu[:], pq[:, 0:T],
                                                BU[:, eo:eo + 1])
                    qv = blate.tile([128, T], BF16, tag=f"qv{eo}", name=f"qv{eo}")
                    nc.gpsimd.tensor_scalar_add(qv[:], qu[:], BDV[:, eo:eo + 1])
                    QU.append(qu)
                    QV.append(qv)
                    pk = pps.tile([128, S2 + 1], F32, tag="pp", name="pk")
                    for ei in range(NEO):
                        nc.tensor.matmul(
                            pk[:, 0:T],
                            WK[:, 512 * ei + 128 * eo:512 * ei + 128 * (eo + 1)],
                            XTs[ei], start=(ei == 0), stop=(ei == NEO - 1))
                    kt = blate.tile([128, T], F32R, tag=f"kt{eo}", name=f"kt{eo}")
                    nc.scalar.activation(kt[:], pk[:, 0:T], AF.Identity,
                                         bias=BK[:, eo:eo + 1])
                    KT.append(kt)
                return QU, QV, KT

            def emit_proj_v(b, XTs):
                """v natural, packed bf16 into 65-col/head layout + ones col."""
                VSB = []
                for st in range(NEO):
                    pv = pps.tile([128, S2 + 1], F32, tag="pp", name="pv")
                    for ei in range(NEO):
                        nc.tensor.matmul(
                            pv[:, 0:E], XTs[ei][:, st * 128:(st + 1) * 128],
                            WV[:, 512 * ei:512 * (ei + 1)],
                            start=(ei == 0), stop=(ei == NEO - 1))
                    vsb = blate.tile([128, W65], BF16, tag=f"v{st}",
                                     name=f"v{st}")
                    nc.vector.tensor_tensor(
                        AP(vsb.tensor, 0, [[W65, 128], [65, H], [1, HD]]),
                        pv[:, 0:E], BVB[:], AT.add)
                    nc.gpsimd.memset(
                        AP(vsb.tensor, HD, [[W65, 128], [65, H]]), 1.0)
                    VSB.append(vsb)
                return VSB

            def emit_proj_p(b, PETs):
                PT = []
                for eo in range(NEO):
                    pp = pps.tile([128, S2 + 1], F32, tag="pp", name="ppp")
                    for ei in range(NEO):
                        nc.tensor.matmul(
                            pp[:, 0:512],
                            WP[:, 512 * ei + 128 * eo:512 * ei + 128 * (eo + 1)],
                            PETs[ei][:, 0:512], start=(ei == 0),
                            stop=(ei == NEO - 1))
                    for ei in range(NEO):
                        nc.tensor.matmul(
                            pp[:, 512:S2 + 1],
                            WP[:, 512 * ei + 128 * eo:512 * ei + 128 * (eo + 1)],
                            PETs[ei][:, 512:S2 + 1], start=(ei == 0),
                            stop=(ei == NEO - 1))
                    pt = blate.tile([128, S2 + 1], BF16, tag=f"pt{eo}",
                                    name=f"pt{eo}")
                    nc.vector.tensor_copy(pt[:], pp[:])
                    PT.append(pt)
                return PT

            def alloc_ubp(hp):
                return [ubp.tile([128, 2 * 640], BF16, tag=f"ub{tt}",
                                 name=f"ub{hp}_{tt}") for tt in range(4)]

            def emit_u_tt(b, h, tt, QV, PT, UBP):
                """one u band matmul group + bf16 copy for (head, t_tile)."""
                hp, i = h // 2, h % 2
                r0 = 64 * (h % 2)
                j0 = J0[tt]
                u = pps.tile([128, S2 + 1], F32, tag="pp", name="u")
                lqv = QV[hp][r0:r0 + 64, 128 * tt:128 * (tt + 1)]
                nc.tensor.matmul(u[:, 0:512], lqv,
                                 PT[hp][r0:r0 + 64, j0:j0 + 512],
                                 start=True, stop=True,
                                 tile_position=(r0, 0))
                nc.tensor.matmul(u[:, 512:640], lqv,
                                 PT[hp][r0:r0 + 64, j0 + 512:j0 + 640],
                                 start=True, stop=True,
                                 tile_position=(r0, 0))
                dst = UBP[tt][:, 640 * i:640 * i + 639]
                if tt == 3:
                    nc.scalar.activation(dst, u[:, 0:639], AF.Copy)
                else:
                    nc.vector.tensor_copy(dst, u[:, 0:639])

            def emit_u_head(b, h, QV, PT, UBP):
                for tt in range(4):
                    emit_u_tt(b, h, tt, QV, PT, UBP)
                return UBP

            def emit_shift(b, hp, UBP):
                """diagonal rel-shift, 2 heads per DMA -> v_pair tiles."""
                VP = []
                for tt in range(4):
                    vp = vpp.tile([128, 2 * T], BF16, tag=f"vp{tt}",
                                  name=f"vp{hp}_{tt}")
                    nc.sync.dma_start(
                        out=vp,
                        in_=AP(UBP[tt].tensor, 127,
                               [[1279, 128], [640, 2], [1, 512]]))
                    VP.append(vp)
                return VP

            def emit_scores(b, h, VP, QU, KT, fillers=()):
                """per s_chunk j: ac^T matmul + 4 shifted-band transposes into
                one PSUM tile, then exp -> bf16 [s,t] tile. `fillers` are
                callables emitting independent PE work (next pair's u groups),
                interleaved after each transpose group so the in-order PE has
                something to chew on while ACT drains the pac buffers."""
                hp, r0, i = h // 2, 64 * (h % 2), h % 2
                fillers = list(fillers)
                PAC = [None] * 4
                ETS = [None] * 4

                def fill():
                    if fillers:
                        fillers.pop(0)()

                def emit_ac(j):
                    pac = pacs.tile([128, T], F32, tag="pac", name="pac")
                    nc.tensor.matmul(pac[:],
                                     KT[hp][r0:r0 + 64, 128 * j:128 * (j + 1)],
                                     QU[hp][r0:r0 + 64, :],
                                     start=True, stop=False,
                                     tile_position=(r0, 0),
                                     skip_group_check=True)
                    PAC[j] = pac

                def emit_texp(j):
                    pac = PAC[j]
                    for tt in range(4):
                        nc.tensor.matmul(
                            pac[:, 128 * tt:128 * (tt + 1)],
                            VP[tt][:, 512 * i + 128 * j:512 * i + 128 * (j + 1)],
                            IDENT[:],
                            start=False, stop=(tt == 3),
                            skip_group_check=True)
                    ets = epp.tile([128, T], BF16, tag=f"e{j}", name=f"e{h}_{j}")
                    nc.scalar.activation(ets[:], pac[:], AF.Exp,
                                         bias=0.0, scale=SCALE)
                    ETS[j] = ets

                emit_ac(0)
                emit_ac(1)
                emit_texp(0)
                fill()
                emit_ac(2)
                emit_texp(1)
                fill()
                emit_ac(3)
                emit_texp(2)
                fill()
                emit_texp(3)
                fill()
                return ETS

            def emit_av(b, h, ETS, VSB):
                """attn @ [v|1] -> [t,65] per tt; evacuate av to SBUF so the
                (PSUM-blind) Pool engine can do the normalize."""
                av = avs.tile([128, 4 * 65], F32, tag="av", name=f"av{h}")
                for tt in range(4):
                    for j in range(4):
                        nc.tensor.matmul(
                            av[:, 65 * tt:65 * (tt + 1)],
                            ETS[j][:, 128 * tt:128 * (tt + 1)],
                            VSB[j][:, 65 * h:65 * (h + 1)],
                            start=(j == 0), stop=(j == 3))
                avc = work.tile([128, 4 * 65], F32, tag="avc", name=f"avc{h}")
                if h % 2 == 0:
                    nc.scalar.activation(avc[:], av[:], AF.Copy)
                else:
                    nc.vector.tensor_copy(avc[:], av[:])
                rec = work.tile([128, 4], F32, tag=f"rec{h}", name=f"rec{h}")
                nc.vector.reciprocal(
                    rec[:], AP(avc.tensor, HD, [[4 * 65, 128], [65, 4]]))
                return avc, rec

            def emit_norm(b, h, avc, rec, OQ):
                c0 = 64 * (h % 2)
                for tt in range(4):
                    nc.gpsimd.tensor_scalar_mul(
                        OQ[:, 128 * tt + c0:128 * tt + c0 + 64],
                        avc[:, 65 * tt:65 * tt + 64], rec[:, tt:tt + 1])

            # ---------------- schedule ----------------
            PET0s = [PET0[:, ei * (S2 + 1):(ei + 1) * (S2 + 1)]
                     for ei in range(NEO)]
            ctx_p = emit_proj_p(0, PET0s)
            ctx_qk = emit_proj_qk(0, XT0)
            ubp_next = alloc_ubp(0)
            emit_u_head(0, 0, ctx_qk[1], ctx_p, ubp_next)
            emit_u_head(0, 1, ctx_qk[1], ctx_p, ubp_next)
            vp_next = emit_shift(0, 0, ubp_next)
            ctx_v = emit_proj_v(0, XT0)
            pend = {}
            for b in range(bpc):
                if b > 0:
                    ctx_qk, ctx_v, ctx_p = pend.pop(b)
                    ubp_next = alloc_ubp(0)
                    emit_u_head(b, 0, ctx_qk[1], ctx_p, ubp_next)
                    emit_u_head(b, 1, ctx_qk[1], ctx_p, ubp_next)
                    vp_next = emit_shift(b, 0, ubp_next)
                QU, QV, KT = ctx_qk
                VSB = ctx_v
                PT = ctx_p
                nxt = {}
                for hp in range(4):
                    VP = vp_next
                    OQ = osb.tile([128, 512], F32, tag=f"o{hp}",
                                  name=f"o{b}_{hp}")
                    # next pair's u matmuls are interleaved INTO the score
                    # sequences as PE filler (the in-order PE otherwise stalls
                    # on pac-buffer reuse at the ACT exp rate); their shifts
                    # are still issued a full phase early.
                    if hp < 3:
                        ubp_n = alloc_ubp(hp + 1)
                        f0 = [(lambda tt=tt: emit_u_tt(
                            b, 2 * (hp + 1), tt, QV, PT, ubp_n))
                            for tt in range(4)]
                        f1 = [(lambda tt=tt: emit_u_tt(
                            b, 2 * (hp + 1) + 1, tt, QV, PT, ubp_n))
                            for tt in range(4)]
                    else:
                        f0, f1 = (), ()
                    ETS0 = emit_scores(b, 2 * hp, VP, QU, KT, fillers=f0)
                    ETS1 = emit_scores(b, 2 * hp + 1, VP, QU, KT, fillers=f1)
                    if hp < 3:
                        vp_next = emit_shift(b, hp + 1, ubp_n)
                    if b + 1 < bpc:
                        if hp == 0:
                            nxt['loads'] = emit_loads(b + 1)
                        elif hp == 1:
                            nxt['p'] = emit_proj_p(b + 1, nxt['loads'][1])
                        elif hp == 2:
                            nxt['qk'] = emit_proj_qk(b + 1, nxt['loads'][0])
                        elif hp == 3:
                            nxt['v'] = emit_proj_v(b + 1, nxt['loads'][0])
                    avc0, rec0 = emit_av(b, 2 * hp, ETS0, VSB)
                    avc1, rec1 = emit_av(b, 2 * hp + 1, ETS1, VSB)
                    emit_norm(b, 2 * hp, avc0, rec0, OQ)
                    emit_norm(b, 2 * hp + 1, avc1, rec1, OQ)
                    # one DMA for the pair's full [T, 128] output column:
                    # src col 128*tt+c -> out[b, 128*tt+p, 128*hp+c]
                    nc.sync.dma_start(
                        out=AP(out_d, b * T * E + 128 * hp,
                               [[512, 128], [65536, 4], [1, 128]]),
                        in_=OQ[:])
                if b + 1 < bpc:
                    pend[b + 1] = (nxt['qk'], nxt['v'], nxt['p'])

    _split_multiwaits(nc, mybir)
    return nc


def _prep_inputs(x, pos_emb, Wq, bq, Wk, bk, Wv, bv, Wp,
                 pos_bias_u, pos_bias_v):
    import ml_dtypes
    BF = ml_dtypes.bfloat16
    xT = np.ascontiguousarray(np.asarray(x, np.float32).transpose(0, 2, 1))
    peT = np.zeros((B, E, S2 + 1), BF)
    peT[:, :, 0:S2] = np.asarray(pos_emb, np.float32).transpose(0, 2, 1).astype(BF)
    wqT = np.ascontiguousarray(np.asarray(Wq, np.float32).T)
    wkT = np.ascontiguousarray(np.asarray(Wk, np.float32).T)
    wvT = np.ascontiguousarray(np.asarray(Wv, np.float32).T)
    wpT = np.ascontiguousarray(np.asarray(Wp, np.float32).T.astype(BF))
    bias_u = (np.asarray(bq, np.float32)
              + np.asarray(pos_bias_u, np.float32).reshape(E))
    bias_v = (np.asarray(bq, np.float32)
              + np.asarray(pos_bias_v, np.float32).reshape(E))
    bu_p = np.ascontiguousarray(bias_u.reshape(4, 128).T)
    bdv_p = np.ascontiguousarray((bias_v - bias_u).reshape(4, 128).T)
    bk_p = np.ascontiguousarray(np.asarray(bk, np.float32).reshape(4, 128).T)
    common = {
        "wqT": wqT, "wkT": wkT, "wvT": wvT, "wpT": wpT,
        "bu": bu_p, "bdv": bdv_p, "bkk": bk_p,
        "bvec": np.asarray(bv, np.float32),
    }
    in_maps = []
    for c in range(N_CORES):
        m = dict(common)
        m["xT"] = xT[c * BPC:(c + 1) * BPC]
        m["peT"] = peT[c * BPC:(c + 1) * BPC]
        in_maps.append(m)
    return in_maps


def kernel(x, pos_emb, Wq, bq, Wk, bk, Wv, bv, Wp,
           pos_bias_u, pos_bias_v, legacy=0, **_):
    from concourse.bass_utils import run_bass_kernel_spmd

    if "nc" not in _CACHE:
        _CACHE["nc"] = _build_nc()
    nc = _CACHE["nc"]
    in_maps = _prep_inputs(x, pos_emb, Wq, bq, Wk, bk, Wv, bv, Wp,
                           pos_bias_u, pos_bias_v)
    res = run_bass_kernel_spmd(nc, in_maps, list(range(N_CORES))).results
    return np.concatenate([r["out"] for r in res], axis=0)   # [B, T, E]



# revision 2
# speedup vs baseline: 1.2012x; 1.2012x over previous
"""Trainium2 Bass kernel for nn_AttentionForONNX (Transformer-XL style
relative-position attention), v2.

Pipeline redesign over v1:
  - PE pstate warmup: dummy matmuls on a memset tile burn the 0.65/1.2 GHz
    ramp while the first DMAs stream, so real matmuls run at 2.4 GHz.
  - x / Wq / Wk / Wv loaded in bf16 (error is dominated by the score-path
    bf16 quantization; measured no change) - halves load DMA.
  - Chunked startup loads interleaved per-ei so the p projection starts
    ~2.5us in instead of 7.6us.
  - Global filler queue: projection eo-groups of the next batch and the
    u-band matmuls of pair k+2 are drained one group at a time into fill
    points inside the score/av phases, keeping the in-order PE busy while
    ACT/DVE drain PSUM.
  - 2-phase u-band lookahead (ubp bufs=3) so each pair's rel-shift DMA is
    issued a full phase early and never gates the transposes.
  - One merged diagonal shift DMA per pair (4x fewer HWDGE slots).
  - All projection/u PSUM tiles are single-bank [128,512] so one 4-deep
    ring + pac(2) + av(2) fits the 8 PSUM banks.
  - Tail: per-head epilogue, norms split Pool/DVE, per-head output DMAs.
"""
import sys
import os

for _p in ("/opt/trn_rl_repo", "/root/.axon_site/_ro/trn_rl_repo"):
    if os.path.isdir(_p) and _p not in sys.path:
        sys.path.insert(0, _p)

import numpy as np

B, T, E, H = 16, 512, 512, 8
HD = E // H
S2 = 2 * T - 1
N_CORES = 8
BPC = B // N_CORES          # batches per core
SCALE = 1.0 / float(np.sqrt(HD))
N_WARM = 10

_CACHE = {}


def _split_multiwaits(nc, mybir):
    """walrus supports only one sync-wait per instruction: split extras
    into single-wait NOPs preceding the instruction."""
    n = 0
    for bb in nc.main_func.blocks:
        new_insts = []
        for ins in bb.instructions:
            si = ins.sync_info
            if si and si.on_wait and len(si.on_wait) > 1:
                waits = list(si.on_wait)
                for w in waits[:-1]:
                    nop = mybir.InstNoOp(name=f"{ins.name}-w{n}", ins=[], outs=[])
                    nop.engine = ins.engine
                    nop.sync_info = mybir.SyncInfo(on_wait=[w], on_update=[])
                    nc.register_instruction(nop, overwrite=True)
                    new_insts.append(nop)
                    n += 1
                ins.sync_info = mybir.SyncInfo(on_wait=[waits[-1]],
                                               on_update=list(si.on_update))
            new_insts.append(ins)
        bb.instructions[:] = new_insts
    return n


def _build_nc(bpc=BPC, n_warm=N_WARM):
    import concourse.bass as bass
    import concourse.mybir as mybir
    import concourse.tile as tile
    from concourse.ap import AP
    from concourse.masks import make_identity

    F32 = mybir.dt.float32
    F32R = mybir.dt.float32r
    BF16 = mybir.dt.bfloat16
    AT = mybir.AluOpType
    AF = mybir.ActivationFunctionType

    nc = bass.Bass("TRN2", target_bir_lowering=False)

    xT = nc.dram_tensor("xT", [bpc, E, T], BF16, kind="ExternalInput")
    peT = nc.dram_tensor("peT", [bpc, E, S2 + 1], BF16, kind="ExternalInput")
    wqT = nc.dram_tensor("wqT", [E, E], BF16, kind="ExternalInput")
    wkT = nc.dram_tensor("wkT", [E, E], BF16, kind="ExternalInput")
    wvT = nc.dram_tensor("wvT", [E, E], BF16, kind="ExternalInput")
    wpT = nc.dram_tensor("wpT", [E, E], BF16, kind="ExternalInput")
    # bias_u / (bias_v - bias_u) / bk packed [128, 4]: col eo = bias[eo*128:+128]
    bu = nc.dram_tensor("bu", [128, 4], F32, kind="ExternalInput")
    bdv = nc.dram_tensor("bdv", [128, 4], F32, kind="ExternalInput")
    bkk = nc.dram_tensor("bkk", [128, 4], F32, kind="ExternalInput")
    bvec = nc.dram_tensor("bvec", [E], F32, kind="ExternalInput")   # bv for v
    out_d = nc.dram_tensor("out", [bpc, T, E], F32, kind="ExternalOutput")

    NEO = E // 128
    J0 = [384 - 128 * tt for tt in range(4)]
    W65 = 65 * H     # 520
    NPAIR = 4 * bpc

    with tile.TileContext(nc) as tc:
        with (
            tc.tile_pool(name="const", bufs=1) as const,
            tc.tile_pool(name="batch", bufs=1) as batch,
            tc.tile_pool(name="blate", bufs=2) as blate,
            tc.tile_pool(name="ubpp", bufs=3) as ubpp,
            tc.tile_pool(name="vpp", bufs=2) as vpp,
            tc.tile_pool(name="epp", bufs=4) as epp,
            tc.tile_pool(name="osb", bufs=2) as osb,
            tc.tile_pool(name="work", bufs=2) as work,
            tc.tile_pool(name="pjp", bufs=4, space="PSUM") as pjp,    # 4 banks
            tc.tile_pool(name="pacs", bufs=2, space="PSUM") as pacs,  # 2 banks
            tc.tile_pool(name="avs", bufs=2, space="PSUM") as avs,    # 2 banks
        ):
            # ---- warmup: no-input matmuls burn the PE pstate ramp while
            # the first loads stream in.
            WARM = const.tile([128, 256], BF16, tag="warm")
            nc.vector.memset(WARM[:], 0.0)
            IDENT = const.tile([128, 128], BF16, tag="ident")
            make_identity(nc, IDENT[:])
            wps = pjp.tile([128, 512], F32, tag="pj", name="warmps")
            for _ in range(n_warm):
                nc.tensor.matmul(wps[:, 0:256], WARM[:, 0:128], WARM[:, 0:256],
                                 start=True, stop=True, skip_group_check=True)

            # ---- startup loads, chunked + interleaved: p path first.
            WP = const.tile([128, 4 * E], BF16, tag="wp")
            PET0 = batch.tile([128, 4 * (S2 + 1)], BF16, tag="pe0", name="peT0")
            XT0 = batch.tile([128, 4 * T], BF16, tag="xt0", name="xT0")
            WQ = const.tile([128, 4 * E], BF16, tag="wq")
            for ei in range(NEO):
                nc.sync.dma_start(
                    out=WP[:, 512 * ei:512 * (ei + 1)],
                    in_=AP(wpT, ei * 65536, [[512, 128], [1, 512]]))
                nc.sync.dma_start(
                    out=PET0[:, 1024 * ei:1024 * (ei + 1)],
                    in_=AP(peT, ei * 131072, [[1024, 128], [1, 1024]]))
            WK = const.tile([128, 4 * E], BF16, tag="wk")
            for c in range(2):
                nc.sync.dma_start(
                    out=XT0[:, 1024 * c:1024 * (c + 1)],
                    in_=AP(xT, c * 131072, [[512, 128], [65536, 2], [1, 512]]))
                nc.sync.dma_start(
                    out=WQ[:, 1024 * c:1024 * (c + 1)],
                    in_=AP(wqT, c * 131072, [[512, 128], [65536, 2], [1, 512]]))
                nc.sync.dma_start(
                    out=WK[:, 1024 * c:1024 * (c + 1)],
                    in_=AP(wkT, c * 131072, [[512, 128], [65536, 2], [1, 512]]))
            BU = const.tile([128, 4], F32, tag="bu")
            nc.sync.dma_start(out=BU, in_=bu[:])
            BDV = const.tile([128, 4], F32, tag="bdv")
            nc.sync.dma_start(out=BDV, in_=bdv[:])
            BK = const.tile([128, 4], F32, tag="bkk")
            nc.sync.dma_start(out=BK, in_=bkk[:])
            BVB = const.tile([128, E], F32, tag="bvb")
            nc.sync.dma_start(out=BVB, in_=AP(bvec, 0, [[0, 128], [1, E]]))
            WV = const.tile([128, 4 * E], BF16, tag="wv")
            nc.sync.dma_start(out=WV,
                              in_=AP(wvT, 0, [[512, 128], [65536, 4], [1, 512]]))

            def xs_of(XTb):
                return [XTb[:, 512 * ei:512 * (ei + 1)] for ei in range(NEO)]

            def ps_of(PETb):
                return [PETb[:, 1024 * ei:1024 * (ei + 1)] for ei in range(NEO)]

            def emit_loads(b):
                XTb = batch.tile([128, 4 * T], BF16, tag=f"xt{b}",
                                 name=f"xT{b}")
                PETb = batch.tile([128, 4 * (S2 + 1)], BF16, tag=f"pe{b}",
                                  name=f"peT{b}")
                for c in range(2):
                    nc.sync.dma_start(
                        out=XTb[:, 1024 * c:1024 * (c + 1)],
                        in_=AP(xT, b * 262144 + c * 131072,
                               [[512, 128], [65536, 2], [1, 512]]))
                for c in range(2):
                    nc.sync.dma_start(
                        out=PETb[:, 2048 * c:2048 * (c + 1)],
                        in_=AP(peT, b * 524288 + c * 262144,
                               [[1024, 128], [131072, 2], [1, 1024]]))
                return xs_of(XTb), ps_of(PETb)

            # ---- per-group projection emitters (each is one filler unit)
            def emit_p_group(b, PETs, eo, half, PT, on_act=False):
                pp = pjp.tile([128, 512], F32, tag="pj", name="pp")
                c0 = 512 * half
                for ei in range(NEO):
                    nc.tensor.matmul(
                        pp[:, 0:512],
                        WP[:, 512 * ei + 128 * eo:512 * ei + 128 * (eo + 1)],
                        PETs[ei][:, c0:c0 + 512],
                        start=(ei == 0), stop=(ei == NEO - 1))
                if PT[eo] is None:
                    PT[eo] = blate.tile([128, S2 + 1], BF16, tag=f"pt{eo}",
                                        name=f"pt{b}_{eo}")
                if on_act:
                    nc.scalar.activation(PT[eo][:, c0:c0 + 512],
                                         pp[:, 0:512], AF.Copy)
                else:
                    nc.vector.tensor_copy(PT[eo][:, c0:c0 + 512], pp[:, 0:512])

            def emit_q_group(b, XTs, eo, QU, QV):
                pq = pjp.tile([128, 512], F32, tag="pj", name="pq")
                for ei in range(NEO):
                    nc.tensor.matmul(
                        pq[:, 0:512],
                        WQ[:, 512 * ei + 128 * eo:512 * ei + 128 * (eo + 1)],
                        XTs[ei], start=(ei == 0), stop=(ei == NEO - 1))
                qu = blate.tile([128, T], F32R, tag=f"qu{eo}", name=f"qu{eo}")
                nc.vector.tensor_scalar_add(qu[:], pq[:, 0:512],
                                            BU[:, eo:eo + 1])
                qv = blate.tile([128, T], BF16, tag=f"qv{eo}", name=f"qv{eo}")
                nc.gpsimd.tensor_scalar_add(qv[:], qu[:], BDV[:, eo:eo + 1])
                QU[eo] = qu
                QV[eo] = qv

            def emit_k_group(b, XTs, eo, KT):
                pk = pjp.tile([128, 512], F32, tag="pj", name="pk")
                for ei in range(NEO):
                    nc.tensor.matmul(
                        pk[:, 0:512],
                        WK[:, 512 * ei + 128 * eo:512 * ei + 128 * (eo + 1)],
                        XTs[ei], start=(ei == 0), stop=(ei == NEO - 1))
                kt = blate.tile([128, T], F32R, tag=f"kt{eo}", name=f"kt{eo}")
                nc.scalar.activation(kt[:], pk[:, 0:512], AF.Identity,
                                     bias=BK[:, eo:eo + 1])
                KT[eo] = kt

            def emit_v_group(b, XTs, st, VSB):
                pv = pjp.tile([128, 512], F32, tag="pj", name="pv")
                for ei in range(NEO):
                    nc.tensor.matmul(
                        pv[:, 0:E], XTs[ei][:, st * 128:(st + 1) * 128],
                        WV[:, 512 * ei:512 * (ei + 1)],
                        start=(ei == 0), stop=(ei == NEO - 1))
                vsb = blate.tile([128, W65], BF16, tag=f"v{st}", name=f"v{st}")
                nc.vector.tensor_tensor(
                    AP(vsb.tensor, 0, [[W65, 128], [65, H], [1, HD]]),
                    pv[:, 0:E], BVB[:], AT.add)
                nc.gpsimd.memset(
                    AP(vsb.tensor, HD, [[W65, 128], [65, H]]), 1.0)
                VSB[st] = vsb

            def emit_u_tt(b, h, tt, QV, PT, UBP, act_units=(0, 6)):
                """u band for (head, t_tile): two 1-bank psum tiles + copies."""
                hp, i = h // 2, h % 2
                r0 = 64 * i
                j0 = J0[tt]
                lqv = QV[hp][r0:r0 + 64, 128 * tt:128 * (tt + 1)]
                ua = pjp.tile([128, 512], F32, tag="pj", name="ua")
                nc.tensor.matmul(ua[:, 0:512], lqv,
                                 PT[hp][r0:r0 + 64, j0:j0 + 512],
                                 start=True, stop=True,
                                 tile_position=(r0, 0))
                ub2 = pjp.tile([128, 512], F32, tag="pj", name="ub2")
                nc.tensor.matmul(ub2[:, 0:128], lqv,
                                 PT[hp][r0:r0 + 64, j0 + 512:j0 + 640],
                                 start=True, stop=True,
                                 tile_position=(r0, 0))
                base = 1280 * tt + 640 * i
                if ((tt << 1) | i) in act_units:
                    nc.scalar.activation(UBP[:, base:base + 512],
                                         ua[:, 0:512], AF.Copy)
                    nc.scalar.activation(UBP[:, base + 512:base + 639],
                                         ub2[:, 0:127], AF.Copy)
                else:
                    nc.vector.tensor_copy(UBP[:, base:base + 512],
                                          ua[:, 0:512])
                    nc.vector.tensor_copy(UBP[:, base + 512:base + 639],
                                          ub2[:, 0:127])

            def alloc_ubp(k):
                return ubpp.tile([128, 4 * 1280], BF16, tag="ub",
                                 name=f"ub{k}")

            def emit_shift(k, UBP):
                """merged diagonal rel-shift DMA for the whole pair."""
                vp = vpp.tile([128, 4096], BF16, tag="vp", name=f"vp{k}")
                nc.sync.dma_start(
                    out=vp,
                    in_=AP(UBP.tensor, 127,
                           [[5119, 128], [1280, 4], [640, 2], [1, 512]]))
                return vp

            fillq = []

            def fill():
                if fillq:
                    fillq.pop(0)()

            def flush():
                while fillq:
                    fillq.pop(0)()

            def emit_scores(b, h, VP, QU, KT):
                hp, r0, i = h // 2, 64 * (h % 2), h % 2
                PAC = [None] * 4
                ETS = [None] * 4

                def emit_ac(j):
                    pac = pacs.tile([128, T], F32, tag="pac", name="pac")
                    nc.tensor.matmul(pac[:],
                                     KT[hp][r0:r0 + 64, 128 * j:128 * (j + 1)],
                                     QU[hp][r0:r0 + 64, :],
                                     start=True, stop=False,
                                     tile_position=(r0, 0),
                                     skip_group_check=True)
                    PAC[j] = pac

                def emit_texp(j):
                    pac = PAC[j]
                    for tt in range(4):
                        nc.tensor.matmul(
                            pac[:, 128 * tt:128 * (tt + 1)],
                            VP[:, 1024 * tt + 512 * i + 128 * j:
                                  1024 * tt + 512 * i + 128 * (j + 1)],
                            IDENT[:],
                            start=False, stop=(tt == 3),
                            skip_group_check=True)
                    ets = epp.tile([128, T], BF16, tag=f"e{j}", name=f"e{h}_{j}")
                    nc.scalar.activation(ets[:], pac[:], AF.Exp,
                                         bias=0.0, scale=SCALE)
                    ETS[j] = ets

                emit_ac(0)
                emit_ac(1)
                emit_texp(0)
                fill()
                emit_ac(2)
                emit_texp(1)
                fill()
                emit_ac(3)
                emit_texp(2)
                fill()
                emit_texp(3)
                fill()
                return ETS

            def emit_av(b, h, ETS, VSB, avc_act=None):
                av = avs.tile([128, 4 * 65], F32, tag="av", name=f"av{h}")
                for tt in range(4):
                    for j in range(4):
                        nc.tensor.matmul(
                            av[:, 65 * tt:65 * (tt + 1)],
                            ETS[j][:, 128 * tt:128 * (tt + 1)],
                            VSB[j][:, 65 * h:65 * (h + 1)],
                            start=(j == 0), stop=(j == 3))
                    fill()
                avc = work.tile([128, 4 * 65], F32, tag="avc", name=f"avc{h}")
                if avc_act is None:
                    avc_act = (h % 2 == 0)
                if avc_act:
                    nc.scalar.activation(avc[:], av[:], AF.Copy)
                else:
                    nc.vector.tensor_copy(avc[:], av[:])
                rec = work.tile([128, 4], F32, tag=f"rec{h}", name=f"rec{h}")
                nc.vector.reciprocal(
                    rec[:], AP(avc.tensor, HD, [[4 * 65, 128], [65, 4]]))
                return avc, rec

            def emit_norm(b, h, avc, rec, OQ, engines=None):
                c0 = 64 * (h % 2)
                for tt in range(4):
                    dst = OQ[:, 128 * tt + c0:128 * tt + c0 + 64]
                    src = avc[:, 65 * tt:65 * tt + 64]
                    if engines and engines[tt] == "dve":
                        nc.vector.tensor_scalar_mul(dst, src, rec[:, tt:tt + 1])
                    else:
                        nc.gpsimd.tensor_scalar_mul(dst, src, rec[:, tt:tt + 1])

            # ================= schedule =================
            # per-batch tile contexts; pair k = (b, hp) = divmod(k, 4)
            ctx = {0: {"QU": [None] * 4, "QV": [None] * 4, "KT": [None] * 4,
                       "PT": [None] * 4, "VSB": [None] * 4,
                       "xs": xs_of(XT0), "ps": ps_of(PET0)}}
            ubp_ring = {}
            vp_ring = {}

            def u_fillers(k):
                """closures for pair k's 8 u_tt groups (order tt-major).
                ACT/DVE copy split tuned per phase load: prologue pairs
                lean on ACT (idle there), late pairs stay 2/8."""
                b, hp = divmod(k, 4)
                ubp_ring[k] = alloc_ubp(k)
                c = ctx[b]
                act_units = (0, 2, 4, 6) if k < 2 else (0, 6)
                res = []
                for tt in range(4):
                    for i in range(2):
                        res.append(lambda tt=tt, i=i, b=b, hp=hp:
                                   emit_u_tt(b, 2 * hp + i, tt,
                                             c["QV"], c["PT"], ubp_ring[k],
                                             act_units))
                return res

            # ---- prologue: batch 0 projections with pair-0/1 u interleave
            # batch-0 p projection, ei-major: each arriving PET/WP chunk is
            # consumed immediately across all four eo tiles (pj ring is 4 deep)
            c0_ = ctx[0]
            for half in range(2):
                c0h = 512 * half
                pph = [pjp.tile([128, 512], F32, tag="pj", name=f"pp{eo}")
                       for eo in range(NEO)]
                for ei in range(NEO):
                    for eo in range(NEO):
                        nc.tensor.matmul(
                            pph[eo][:, 0:512],
                            WP[:, 512 * ei + 128 * eo:512 * ei + 128 * (eo + 1)],
                            c0_["ps"][ei][:, c0h:c0h + 512],
                            start=(ei == 0), stop=(ei == NEO - 1),
                            skip_group_check=True)
                for eo in range(NEO):
                    if c0_["PT"][eo] is None:
                        c0_["PT"][eo] = blate.tile([128, S2 + 1], BF16,
                                                   tag=f"pt{eo}",
                                                   name=f"pt0_{eo}")
                    if half == 1:
                        nc.scalar.activation(
                            c0_["PT"][eo][:, c0h:c0h + 512],
                            pph[eo][:, 0:512], AF.Copy)
                    else:
                        nc.vector.tensor_copy(
                            c0_["PT"][eo][:, c0h:c0h + 512], pph[eo][:, 0:512])
            # pair-0 u fillers wait on the qu->qv chain; emit the first q/k
            # groups before draining any so the chain has latency cover.
            emit_q_group(0, c0_["xs"], 0, c0_["QU"], c0_["QV"])
            emit_k_group(0, c0_["xs"], 0, c0_["KT"])
            fillq.extend(u_fillers(0))
            for eo in range(1, NEO):
                emit_q_group(0, c0_["xs"], eo, c0_["QU"], c0_["QV"])
                fill()
                fill()
                emit_k_group(0, c0_["xs"], eo, c0_["KT"])
                fill()
            flush()
            vp_ring[0] = emit_shift(0, ubp_ring[0])
            fillq.extend(u_fillers(1))
            for st in range(NEO):
                emit_v_group(0, c0_["xs"], st, c0_["VSB"])
                fill()
                fill()
            flush()

            def finish_pair(b, hp, ETS0, ETS1, last):
                """av + normalize + output DMA for pair (b, hp) — emitted one
                phase late so the exp chain never gates the av matmuls."""
                cb = ctx[b]
                OQ = osb.tile([128, 512], F32, tag="oq", name=f"o{b}_{hp}")
                if not last:
                    avc0, rec0 = emit_av(b, 2 * hp, ETS0, cb["VSB"])
                    avc1, rec1 = emit_av(b, 2 * hp + 1, ETS1, cb["VSB"])
                    emit_norm(b, 2 * hp, avc0, rec0, OQ)
                    emit_norm(b, 2 * hp + 1, avc1, rec1, OQ)
                    nc.sync.dma_start(
                        out=AP(out_d, b * T * E + 128 * hp,
                               [[512, 128], [65536, 4], [1, 128]]),
                        in_=OQ[:])
                else:
                    # tail: per-head epilogue, avc on ACT, norms on DVE,
                    # split per-head DMAs
                    avc0, rec0 = emit_av(b, 2 * hp, ETS0, cb["VSB"],
                                         avc_act=True)
                    emit_norm(b, 2 * hp, avc0, rec0, OQ,
                              engines=("dve", "dve", "dve", "dve"))
                    nc.sync.dma_start(
                        out=AP(out_d, b * T * E + 128 * hp,
                               [[512, 128], [65536, 4], [1, 64]]),
                        in_=AP(OQ.tensor, 0, [[512, 128], [128, 4], [1, 64]]))
                    avc1, rec1 = emit_av(b, 2 * hp + 1, ETS1, cb["VSB"],
                                         avc_act=True)
                    emit_norm(b, 2 * hp + 1, avc1, rec1, OQ,
                              engines=("dve", "dve", "dve", "dve"))
                    nc.sync.dma_start(
                        out=AP(out_d, b * T * E + 128 * hp + 64,
                               [[512, 128], [65536, 4], [1, 64]]),
                        in_=AP(OQ.tensor, 64, [[512, 128], [128, 4], [1, 64]]))

            # ---- main pair loop (av stage pipelined one phase behind)
            pending_av = None
            for k in range(NPAIR):
                b, hp = divmod(k, 4)
                last = (k == NPAIR - 1)
                cb = ctx[b]
                if k + 1 < NPAIR:
                    vp_ring[k + 1] = emit_shift(k + 1, ubp_ring[k + 1])
                if hp == 0 and b + 1 < bpc:
                    nb = {"QU": [None] * 4, "QV": [None] * 4,
                          "KT": [None] * 4, "PT": [None] * 4,
                          "VSB": [None] * 4}
                    nb["xs"], nb["ps"] = emit_loads(b + 1)
                    ctx[b + 1] = nb
                # fillers for this phase
                if k + 2 < NPAIR:
                    uf = u_fillers(k + 2)
                else:
                    uf = []
                pf = []
                if b + 1 < bpc:
                    nb = ctx[b + 1]
                    if hp == 1:
                        pf = [
                            (lambda eo=eo, half=half:
                             emit_p_group(b + 1, nb["ps"], eo, half, nb["PT"]))
                            for eo in range(NEO) for half in range(2)]
                    elif hp == 2:
                        for eo in range(NEO):
                            pf.append(lambda eo=eo:
                                      emit_q_group(b + 1, nb["xs"], eo,
                                                   nb["QU"], nb["QV"]))
                            pf.append(lambda eo=eo:
                                      emit_k_group(b + 1, nb["xs"], eo,
                                                   nb["KT"]))
                    elif hp == 3:
                        pf = [(lambda st=st:
                               emit_v_group(b + 1, nb["xs"], st, nb["VSB"]))
                              for st in range(NEO)]
                if hp == 2:
                    fillq.extend(pf)      # qk must precede next batch's u
                    fillq.extend(uf)
                else:
                    fillq.extend(uf)
                    fillq.extend(pf)

                VP = vp_ring.pop(k)
                ETS0 = emit_scores(b, 2 * hp, VP, cb["QU"], cb["KT"])
                ETS1 = emit_scores(b, 2 * hp + 1, VP, cb["QU"], cb["KT"])
                if pending_av is not None:
                    finish_pair(*pending_av, last=False)
                pending_av = (b, hp, ETS0, ETS1)
                flush()
            finish_pair(*pending_av, last=True)

    _split_multiwaits(nc, mybir)
    return nc


def _prep_inputs(x, pos_emb, Wq, bq, Wk, bk, Wv, bv, Wp,
                 pos_bias_u, pos_bias_v):
    import ml_dtypes
    BF = ml_dtypes.bfloat16
    xT = np.ascontiguousarray(
        np.asarray(x, np.float32).transpose(0, 2, 1)).astype(BF)
    peT = np.zeros((B, E, S2 + 1), BF)
    peT[:, :, 0:S2] = np.asarray(pos_emb, np.float32).transpose(0, 2, 1).astype(BF)
    wqT = np.ascontiguousarray(np.asarray(Wq, np.float32).T.astype(BF))
    wkT = np.ascontiguousarray(np.asarray(Wk, np.float32).T.astype(BF))
    wvT = np.ascontiguousarray(np.asarray(Wv, np.float32).T.astype(BF))
    wpT = np.ascontiguousarray(np.asarray(Wp, np.float32).T.astype(BF))
    bias_u = (np.asarray(bq, np.float32)
              + np.asarray(pos_bias_u, np.float32).reshape(E))
    bias_v = (np.asarray(bq, np.float32)
              + np.asarray(pos_bias_v, np.float32).reshape(E))
    bu_p = np.ascontiguousarray(bias_u.reshape(4, 128).T)
    bdv_p = np.ascontiguousarray((bias_v - bias_u).reshape(4, 128).T)
    bk_p = np.ascontiguousarray(np.asarray(bk, np.float32).reshape(4, 128).T)
    common = {
        "wqT": wqT, "wkT": wkT, "wvT": wvT, "wpT": wpT,
        "bu": bu_p, "bdv": bdv_p, "bkk": bk_p,
        "bvec": np.asarray(bv, np.float32),
    }
    in_maps = []
    for c in range(N_CORES):
        m = dict(common)
        m["xT"] = xT[c * BPC:(c + 1) * BPC]
        m["peT"] = peT[c * BPC:(c + 1) * BPC]
        in_maps.append(m)
    return in_maps


def kernel(x, pos_emb, Wq, bq, Wk, bk, Wv, bv, Wp,
           pos_bias_u, pos_bias_v, legacy=0, **_):
    from concourse.bass_utils import run_bass_kernel_spmd

    if "nc" not in _CACHE:
        _CACHE["nc"] = _build_nc()
    nc = _CACHE["nc"]
    in_maps = _prep_inputs(x, pos_emb, Wq, bq, Wk, bk, Wv, bv, Wp,
                           pos_bias_u, pos_bias_v)
    res = run_bass_kernel_spmd(nc, in_maps, list(range(N_CORES))).results
    return np.concatenate([r["out"] for r in res], axis=0)   # [B, T, E]


# revision 4
# speedup vs baseline: 1.2621x; 1.0507x over previous
"""Trainium2 Bass kernel for nn_AttentionForONNX (Transformer-XL style
relative-position attention), v2.

Pipeline redesign over v1 (133.5us -> 105.8us in the TimelineSim cost
model):
  - PE pstate warmup: dummy matmuls on a scratch tile burn the 0.65/1.2
    GHz ramp while the first DMAs stream, so real matmuls run at 2.4 GHz.
  - x / Wq / Wk / Wv loaded in bf16 (output error is dominated by the
    score-path bf16 quantization; measured no change) - halves load DMA.
  - Chunked startup loads interleaved per-ei, and the batch-0 p projection
    runs ei-major so each arriving chunk is consumed immediately.
  - Global filler queue: projection eo-groups of the next batch and the
    u-band matmuls of pair k+2 are drained one group at a time into fill
    points inside the score/av phases, keeping the in-order PE busy while
    ACT/DVE drain PSUM.
  - 2-phase u-band lookahead (ubp bufs=3) so each pair's rel-shift DMA is
    issued a full phase early and never gates the transposes.
  - One merged diagonal shift DMA per pair (4x fewer HWDGE slots).
  - av/normalize stage software-pipelined one phase behind its scores so
    the ACT exp chain never gates the av matmuls; the last pair's scores
    are pre-emitted on the then-idle u-band PSUM ring for the same reason.
  - PSUM: u-band pool (2x2 banks) + shared 1-bank proj/av ring (2) +
    pac ring (2) = 8 banks.
  - Tail: per-head epilogue, avc on ACT, norms on DVE, per-head DMAs.
"""
import sys
import os

for _p in ("/opt/trn_rl_repo", "/root/.axon_site/_ro/trn_rl_repo"):
    if os.path.isdir(_p) and _p not in sys.path:
        sys.path.insert(0, _p)

import numpy as np

B, T, E, H = 16, 512, 512, 8
HD = E // H
S2 = 2 * T - 1
N_CORES = 8
BPC = B // N_CORES          # batches per core
SCALE = 1.0 / float(np.sqrt(HD))
N_WARM = 10

_CACHE = {}


def _split_multiwaits(nc, mybir):
    """walrus supports only one sync-wait per instruction: split extras
    into single-wait NOPs preceding the instruction."""
    n = 0
    for bb in nc.main_func.blocks:
        new_insts = []
        for ins in bb.instructions:
            si = ins.sync_info
            if si and si.on_wait and len(si.on_wait) > 1:
                waits = list(si.on_wait)
                for w in waits[:-1]:
                    nop = mybir.InstNoOp(name=f"{ins.name}-w{n}", ins=[], outs=[])
                    nop.engine = ins.engine
                    nop.sync_info = mybir.SyncInfo(on_wait=[w], on_update=[])
                    nc.register_instruction(nop, overwrite=True)
                    new_insts.append(nop)
                    n += 1
                ins.sync_info = mybir.SyncInfo(on_wait=[waits[-1]],
                                               on_update=list(si.on_update))
            new_insts.append(ins)
        bb.instructions[:] = new_insts
    return n


def _build_nc(bpc=BPC, n_warm=N_WARM, epp_bufs=4, ubpp_bufs=3, am0=0, am1=6, qu_act0=0, qu_act1=0):
    import concourse.bass as bass
    import concourse.mybir as mybir
    import concourse.tile as tile
    from concourse.ap import AP
    from concourse.masks import make_identity

    F32 = mybir.dt.float32
    F32R = mybir.dt.float32r
    BF16 = mybir.dt.bfloat16
    AT = mybir.AluOpType
    AF = mybir.ActivationFunctionType

    nc = bass.Bass("TRN2", target_bir_lowering=False)

    xT = nc.dram_tensor("xT", [bpc, E, T], BF16, kind="ExternalInput")
    peT = nc.dram_tensor("peT", [bpc, E, S2 + 1], BF16, kind="ExternalInput")
    wqT = nc.dram_tensor("wqT", [E, E], BF16, kind="ExternalInput")
    wkT = nc.dram_tensor("wkT", [E, E], BF16, kind="ExternalInput")
    wvT = nc.dram_tensor("wvT", [E, E], BF16, kind="ExternalInput")
    wpT = nc.dram_tensor("wpT", [E, E], BF16, kind="ExternalInput")
    # bias_u / (bias_v - bias_u) / bk packed [128, 4]: col eo = bias[eo*128:+128]
    bu = nc.dram_tensor("bu", [128, 4], F32, kind="ExternalInput")
    bdv = nc.dram_tensor("bdv", [128, 4], F32, kind="ExternalInput")
    bvp = nc.dram_tensor("bvp", [128, 4], F32, kind="ExternalInput")
    bkk = nc.dram_tensor("bkk", [128, 4], F32, kind="ExternalInput")
    bvec = nc.dram_tensor("bvec", [E], F32, kind="ExternalInput")   # bv for v
    out_d = nc.dram_tensor("out", [bpc, T, E], F32, kind="ExternalOutput")

    NEO = E // 128
    J0 = [384 - 128 * tt for tt in range(4)]
    W65 = 65 * H     # 520
    NPAIR = 4 * bpc

    with tile.TileContext(nc) as tc:
        with (
            tc.tile_pool(name="const", bufs=1) as const,
            tc.tile_pool(name="batch", bufs=1) as batch,
            tc.tile_pool(name="blate", bufs=2) as blate,
            tc.tile_pool(name="ubpp", bufs=ubpp_bufs) as ubpp,
            tc.tile_pool(name="vpp", bufs=2) as vpp,
            tc.tile_pool(name="epp", bufs=epp_bufs) as epp,
            tc.tile_pool(name="osb", bufs=2) as osb,
            tc.tile_pool(name="work", bufs=2) as work,
            tc.tile_pool(name="pup", bufs=2, space="PSUM") as pup,    # 4 banks
            tc.tile_pool(name="pjp", bufs=2, space="PSUM") as pjp,    # 2 banks
            tc.tile_pool(name="pacs", bufs=2, space="PSUM") as pacs,  # 2 banks
        ):
            # ---- warmup: matmuls on an uninitialized scratch tile burn the
            # PE pstate ramp while the first loads stream in. The psum
            # result is never read, so the garbage input is harmless, and
            # skipping the memset lets the PE start ~70ns in.
            WARM = const.tile([128, 257], BF16, tag="warm")
            nc.vector.memset(WARM[:, 256:257], 0.0)
            IDENT = const.tile([128, 128], BF16, tag="ident")
            make_identity(nc, IDENT[:])
            wps = pjp.tile([128, 512], F32, tag="pj", name="warmps")
            for _ in range(n_warm):
                nc.tensor.matmul(wps[:, 0:256], WARM[:, 0:128], WARM[:, 0:256],
                                 start=True, stop=True, skip_group_check=True)

            # ---- startup loads, chunked + interleaved: p path first.
            WP = const.tile([128, 4 * E], BF16, tag="wp")
            PET0 = batch.tile([128, 4 * (S2 + 1)], BF16, tag="pe0", name="peT0")
            XT0 = batch.tile([128, 4 * T], BF16, tag="xt0", name="xT0")
            WQ = const.tile([128, 4 * E], BF16, tag="wq")
            BU = const.tile([128, 4], F32, tag="bu")
            for ei in range(NEO):
                nc.sync.dma_start(
                    out=WP[:, 512 * ei:512 * (ei + 1)],
                    in_=AP(wpT, ei * 65536, [[512, 128], [1, 512]]))
                nc.sync.dma_start(
                    out=PET0[:, 1024 * ei:1024 * (ei + 1)],
                    in_=AP(peT, ei * 131072, [[1024, 128], [1, 1024]]))

            WK = const.tile([128, 4 * E], BF16, tag="wk")
            for c in range(2):
                nc.sync.dma_start(
                    out=XT0[:, 1024 * c:1024 * (c + 1)],
                    in_=AP(xT, c * 131072, [[512, 128], [65536, 2], [1, 512]]))
                nc.sync.dma_start(
                    out=WQ[:, 1024 * c:1024 * (c + 1)],
                    in_=AP(wqT, c * 131072, [[512, 128], [65536, 2], [1, 512]]))
                nc.sync.dma_start(
                    out=WK[:, 1024 * c:1024 * (c + 1)],
                    in_=AP(wkT, c * 131072, [[512, 128], [65536, 2], [1, 512]]))
            nc.sync.dma_start(out=BU, in_=bu[:])
            BDV = const.tile([128, 4], F32, tag="bdv")
            nc.sync.dma_start(out=BDV, in_=bdv[:])
            BK = const.tile([128, 4], F32, tag="bkk")
            nc.sync.dma_start(out=BK, in_=bkk[:])
            BVB = const.tile([128, E], F32, tag="bvb")
            nc.sync.dma_start(out=BVB, in_=AP(bvec, 0, [[0, 128], [1, E]]))
            WV = const.tile([128, 4 * E], BF16, tag="wv")
            nc.sync.dma_start(out=WV,
                              in_=AP(wvT, 0, [[512, 128], [65536, 4], [1, 512]]))

            def xs_of(XTb):
                return [XTb[:, 512 * ei:512 * (ei + 1)] for ei in range(NEO)]

            def ps_of(PETb):
                return [PETb[:, 1024 * ei:1024 * (ei + 1)] for ei in range(NEO)]

            def emit_loads(b):
                XTb = batch.tile([128, 4 * T], BF16, tag=f"xt{b}",
                                 name=f"xT{b}")
                PETb = batch.tile([128, 4 * (S2 + 1)], BF16, tag=f"pe{b}",
                                  name=f"peT{b}")
                for c in range(2):
                    nc.sync.dma_start(
                        out=XTb[:, 1024 * c:1024 * (c + 1)],
                        in_=AP(xT, b * 262144 + c * 131072,
                               [[512, 128], [65536, 2], [1, 512]]))
                for c in range(2):
                    nc.sync.dma_start(
                        out=PETb[:, 2048 * c:2048 * (c + 1)],
                        in_=AP(peT, b * 524288 + c * 262144,
                               [[1024, 128], [131072, 2], [1, 1024]]))
                return xs_of(XTb), ps_of(PETb)

            # ---- per-group projection emitters (each is one filler unit)
            def emit_p_group(b, PETs, eo, half, PT, on_act=False):
                pp = pjp.tile([128, 512], F32, tag="pj", name="pp")
                c0 = 512 * half
                for ei in range(NEO):
                    nc.tensor.matmul(
                        pp[:, 0:512],
                        WP[:, 512 * ei + 128 * eo:512 * ei + 128 * (eo + 1)],
                        PETs[ei][:, c0:c0 + 512],
                        start=(ei == 0), stop=(ei == NEO - 1))
                if PT[eo] is None:
                    PT[eo] = blate.tile([128, S2 + 1], BF16, tag=f"pt{eo}",
                                        name=f"pt{b}_{eo}")
                if on_act:
                    nc.scalar.activation(PT[eo][:, c0:c0 + 512],
                                         pp[:, 0:512], AF.Copy)
                else:
                    nc.vector.tensor_copy(PT[eo][:, c0:c0 + 512], pp[:, 0:512])

            def emit_q_group(b, XTs, eo, QU, QV, qv_act=False):
                qu_act = qu_act0 if b == 0 else qu_act1
                pq = pjp.tile([128, 512], F32, tag="pj", name="pq")
                for ei in range(NEO):
                    nc.tensor.matmul(
                        pq[:, 0:512],
                        WQ[:, 512 * ei + 128 * eo:512 * ei + 128 * (eo + 1)],
                        XTs[ei], start=(ei == 0), stop=(ei == NEO - 1))
                qu = blate.tile([128, T], F32R, tag=f"qu{eo}", name=f"qu{eo}")
                if qu_act:
                    nc.scalar.activation(qu[:], pq[:, 0:512], AF.Identity,
                                         bias=BU[:, eo:eo + 1])
                else:
                    nc.vector.tensor_scalar_add(qu[:], pq[:, 0:512],
                                                BU[:, eo:eo + 1])
                qv = blate.tile([128, T], BF16, tag=f"qv{eo}", name=f"qv{eo}")
                nc.gpsimd.tensor_scalar_add(qv[:], qu[:],
                                            BDV[:, eo:eo + 1])
                QU[eo] = qu
                QV[eo] = qv

            def emit_k_group(b, XTs, eo, KT):
                pk = pjp.tile([128, 512], F32, tag="pj", name="pk")
                for ei in range(NEO):
                    nc.tensor.matmul(
                        pk[:, 0:512],
                        WK[:, 512 * ei + 128 * eo:512 * ei + 128 * (eo + 1)],
                        XTs[ei], start=(ei == 0), stop=(ei == NEO - 1))
                kt = blate.tile([128, T], F32R, tag=f"kt{eo}", name=f"kt{eo}")
                nc.scalar.activation(kt[:], pk[:, 0:512], AF.Identity,
                                     bias=BK[:, eo:eo + 1])
                KT[eo] = kt

            def emit_v_group(b, XTs, st, VSB):
                pv = pjp.tile([128, 512], F32, tag="pj", name="pv")
                for ei in range(NEO):
                    nc.tensor.matmul(
                        pv[:, 0:E], XTs[ei][:, st * 128:(st + 1) * 128],
                        WV[:, 512 * ei:512 * (ei + 1)],
                        start=(ei == 0), stop=(ei == NEO - 1))
                vsb = blate.tile([128, W65], BF16, tag=f"v{st}", name=f"v{st}")
                nc.vector.tensor_tensor(
                    AP(vsb.tensor, 0, [[W65, 128], [65, H], [1, HD]]),
                    pv[:, 0:E], BVB[:], AT.add)
                nc.gpsimd.memset(
                    AP(vsb.tensor, HD, [[W65, 128], [65, H]]), 1.0)
                VSB[st] = vsb

            def emit_u_tt(b, h, tt, QV, PT, UBP, act_units=(0, 6)):
                """u band for (head, t_tile): one 2-bank psum tile, one copy."""
                hp, i = h // 2, h % 2
                r0 = 64 * i
                j0 = J0[tt]
                lqv = QV[hp][r0:r0 + 64, 128 * tt:128 * (tt + 1)]
                ua = pup.tile([128, 640], F32, tag="pu", name="ua")
                nc.tensor.matmul(ua[:, 0:512], lqv,
                                 PT[hp][r0:r0 + 64, j0:j0 + 512],
                                 start=True, stop=True,
                                 tile_position=(r0, 0))
                nc.tensor.matmul(ua[:, 512:640], lqv,
                                 PT[hp][r0:r0 + 64, j0 + 512:j0 + 640],
                                 start=True, stop=True,
                                 tile_position=(r0, 0))
                base = 1280 * tt + 640 * i
                if ((tt << 1) | i) in act_units:
                    nc.scalar.activation(UBP[:, base:base + 639],
                                         ua[:, 0:639], AF.Copy)
                else:
                    nc.vector.tensor_copy(UBP[:, base:base + 639],
                                          ua[:, 0:639])

            def alloc_ubp(k):
                return ubpp.tile([128, 4 * 1280], BF16, tag="ub",
                                 name=f"ub{k}")

            def emit_shift(k, UBP, split=False):
                """diagonal rel-shift DMA; split per-tt for the startup pairs
                whose shift is on the critical path."""
                vp = vpp.tile([128, 4096], BF16, tag="vp", name=f"vp{k}")
                if split:
                    for tt in range(4):
                        nc.sync.dma_start(
                            out=vp[:, 1024 * tt:1024 * (tt + 1)],
                            in_=AP(UBP.tensor, 127 + 1280 * tt,
                                   [[5119, 128], [640, 2], [1, 512]]))
                else:
                    nc.sync.dma_start(
                        out=vp,
                        in_=AP(UBP.tensor, 127,
                               [[5119, 128], [1280, 4], [640, 2], [1, 512]]))
                return vp

            fillq = []

            def fill():
                if fillq:
                    fillq.pop(0)()

            def flush():
                while fillq:
                    fillq.pop(0)()

            def emit_scores(b, h, VP, QU, KT, pac_pool=None, pac_tag="pac"):
                hp, r0, i = h // 2, 64 * (h % 2), h % 2
                pool = pac_pool if pac_pool is not None else pacs
                PAC = [None] * 4
                ETS = [None] * 4

                def emit_ac(j):
                    pac = pool.tile([128, T], F32, tag=pac_tag, name="pac")
                    nc.tensor.matmul(pac[:],
                                     KT[hp][r0:r0 + 64, 128 * j:128 * (j + 1)],
                                     QU[hp][r0:r0 + 64, :],
                                     start=True, stop=False,
                                     tile_position=(r0, 0),
                                     skip_group_check=True)
                    PAC[j] = pac

                def emit_texp(j):
                    pac = PAC[j]
                    for tt in range(4):
                        nc.tensor.matmul(
                            pac[:, 128 * tt:128 * (tt + 1)],
                            VP[:, 1024 * tt + 512 * i + 128 * j:
                                  1024 * tt + 512 * i + 128 * (j + 1)],
                            IDENT[:],
                            start=False, stop=(tt == 3),
                            skip_group_check=True)
                    ets = epp.tile([128, T], BF16, tag=f"e{j}", name=f"e{h}_{j}")
                    nc.scalar.activation(ets[:], pac[:], AF.Exp,
                                         bias=0.0, scale=SCALE)
                    ETS[j] = ets

                emit_ac(0)
                emit_ac(1)
                emit_texp(0)
                fill()
                emit_ac(2)
                emit_texp(1)
                fill()
                emit_ac(3)
                emit_texp(2)
                fill()
                emit_texp(3)
                fill()
                return ETS

            def emit_av(b, h, ETS, VSB, avc_act=None):
                av = pjp.tile([128, 4 * 65], F32, tag="pj", name=f"av{h}")
                for tt in range(4):
                    for j in range(4):
                        nc.tensor.matmul(
                            av[:, 65 * tt:65 * (tt + 1)],
                            ETS[j][:, 128 * tt:128 * (tt + 1)],
                            VSB[j][:, 65 * h:65 * (h + 1)],
                            start=(j == 0), stop=(j == 3))
                    fill()
                avc = work.tile([128, 4 * 65], F32, tag="avc", name=f"avc{h}")
                if avc_act is None:
                    avc_act = (h % 2 == 0)
                if avc_act:
                    nc.scalar.activation(avc[:], av[:], AF.Copy)
                else:
                    nc.vector.tensor_copy(avc[:], av[:])
                rec = work.tile([128, 4], F32, tag=f"rec{h}", name=f"rec{h}")
                nc.vector.reciprocal(
                    rec[:], AP(avc.tensor, HD, [[4 * 65, 128], [65, 4]]))
                return avc, rec

            def emit_norm(b, h, avc, rec, OQ, engines=None):
                c0 = 64 * (h % 2)
                for tt in range(4):
                    dst = OQ[:, 128 * tt + c0:128 * tt + c0 + 64]
                    src = avc[:, 65 * tt:65 * tt + 64]
                    if engines and engines[tt] == "dve":
                        nc.vector.tensor_scalar_mul(dst, src, rec[:, tt:tt + 1])
                    else:
                        nc.gpsimd.tensor_scalar_mul(dst, src, rec[:, tt:tt + 1])

            # ================= schedule =================
            # per-batch tile contexts; pair k = (b, hp) = divmod(k, 4)
            ctx = {0: {"QU": [None] * 4, "QV": [None] * 4, "KT": [None] * 4,
                       "PT": [None] * 4, "VSB": [None] * 4,
                       "xs": xs_of(XT0), "ps": ps_of(PET0)}}
            ubp_ring = {}
            vp_ring = {}

            def u_fillers(k):
                """closures for pair k's 8 u_tt groups (order tt-major).
                ACT/DVE copy split tuned per phase load: prologue pairs
                lean on ACT (idle there), late pairs stay 2/8."""
                b, hp = divmod(k, 4)
                ubp_ring[k] = alloc_ubp(k)
                c = ctx[b]
                act_units = (0, 2, 4, 6) if k < 2 else tuple(u for u in (am0, am1) if u >= 0)
                res = []
                for tt in range(4):
                    for i in range(2):
                        res.append(lambda tt=tt, i=i, b=b, hp=hp:
                                   emit_u_tt(b, 2 * hp + i, tt,
                                             c["QV"], c["PT"], ubp_ring[k],
                                             act_units))
                return res

            # ---- prologue: batch 0 projections with pair-0/1 u interleave
            # batch-0 p projection, ei-major: each arriving PET/WP chunk is
            # consumed immediately across all four eo tiles (pj ring is 4 deep)
            c0_ = ctx[0]
            for half in range(2):
                c0h = 512 * half
                pph = [(pup if eo < 2 else pjp).tile(
                    [128, 640 if eo < 2 else 512], F32,
                    tag="pu" if eo < 2 else "pj", name=f"pp{eo}")
                       for eo in range(NEO)]
                for ei in range(NEO):
                    for eo in range(NEO):
                        nc.tensor.matmul(
                            pph[eo][:, 0:512],
                            WP[:, 512 * ei + 128 * eo:512 * ei + 128 * (eo + 1)],
                            c0_["ps"][ei][:, c0h:c0h + 512],
                            start=(ei == 0), stop=(ei == NEO - 1),
                            skip_group_check=True)
                for eo in range(NEO):
                    if c0_["PT"][eo] is None:
                        c0_["PT"][eo] = blate.tile([128, S2 + 1], BF16,
                                                   tag=f"pt{eo}",
                                                   name=f"pt0_{eo}")
                    if half == 1:
                        nc.scalar.activation(
                            c0_["PT"][eo][:, c0h:c0h + 512],
                            pph[eo][:, 0:512], AF.Copy)
                    else:
                        nc.vector.tensor_copy(
                            c0_["PT"][eo][:, c0h:c0h + 512], pph[eo][:, 0:512])
            # pair-0 u fillers wait on the qu->qv chain; emit the first q/k
            # groups before draining any so the chain has latency cover.
            emit_q_group(0, c0_["xs"], 0, c0_["QU"], c0_["QV"])
            emit_k_group(0, c0_["xs"], 0, c0_["KT"])
            fillq.extend(u_fillers(0))
            for eo in range(1, NEO):
                emit_q_group(0, c0_["xs"], eo, c0_["QU"], c0_["QV"])
                fill()
                fill()
                emit_k_group(0, c0_["xs"], eo, c0_["KT"])
                fill()
            flush()
            vp_ring[0] = emit_shift(0, ubp_ring[0])
            fillq.extend(u_fillers(1))
            for st in range(NEO):
                emit_v_group(0, c0_["xs"], st, c0_["VSB"])
                fill()
                fill()
            flush()

            def finish_pair(b, hp, ETS0, ETS1, last):
                """av + normalize + output DMA for pair (b, hp) — emitted one
                phase late so the exp chain never gates the av matmuls."""
                cb = ctx[b]
                OQ = osb.tile([128, 512], F32, tag="oq", name=f"o{b}_{hp}")
                late = 4 * b + hp >= 5
                if not last:
                    avc0, rec0 = emit_av(b, 2 * hp, ETS0, cb["VSB"],
                                         avc_act=False if late else None)
                    avc1, rec1 = emit_av(b, 2 * hp + 1, ETS1, cb["VSB"],
                                         avc_act=False if late else None)
                    emit_norm(b, 2 * hp, avc0, rec0, OQ)
                    emit_norm(b, 2 * hp + 1, avc1, rec1, OQ)
                    nc.sync.dma_start(
                        out=AP(out_d, b * T * E + 128 * hp,
                               [[512, 128], [65536, 4], [1, 128]]),
                        in_=OQ[:])
                else:
                    # tail: per-head epilogue, avc on ACT, norms on DVE,
                    # split per-head DMAs
                    avc0, rec0 = emit_av(b, 2 * hp, ETS0, cb["VSB"],
                                         avc_act=True)
                    emit_norm(b, 2 * hp, avc0, rec0, OQ,
                              engines=("dve", "dve", "dve", "dve"))
                    nc.sync.dma_start(
                        out=AP(out_d, b * T * E + 128 * hp,
                               [[512, 128], [65536, 4], [1, 64]]),
                        in_=AP(OQ.tensor, 0, [[512, 128], [128, 4], [1, 64]]))
                    avc1, rec1 = emit_av(b, 2 * hp + 1, ETS1, cb["VSB"],
                                         avc_act=True)
                    emit_norm(b, 2 * hp + 1, avc1, rec1, OQ,
                              engines=("dve", "dve", "dve", "dve"))
                    nc.sync.dma_start(
                        out=AP(out_d, b * T * E + 128 * hp + 64,
                               [[512, 128], [65536, 4], [1, 64]]),
                        in_=AP(OQ.tensor, 64, [[512, 128], [128, 4], [1, 64]]))

            # ---- main pair loop (av stage pipelined one phase behind)
            pending_av = None
            sc_cache = {}
            for k in range(NPAIR):
                b, hp = divmod(k, 4)
                last = (k == NPAIR - 1)
                cb = ctx[b]
                if k + 1 < NPAIR:
                    vp_ring[k + 1] = emit_shift(k + 1, ubp_ring[k + 1])
                if hp == 0 and b + 1 < bpc:
                    nb = {"QU": [None] * 4, "QV": [None] * 4,
                          "KT": [None] * 4, "PT": [None] * 4,
                          "VSB": [None] * 4}
                    nb["xs"], nb["ps"] = emit_loads(b + 1)
                    ctx[b + 1] = nb
                # fillers for this phase
                if k + 2 < NPAIR:
                    uf = u_fillers(k + 2)
                else:
                    uf = []
                pf = []
                if b + 1 < bpc:
                    nb = ctx[b + 1]
                    if hp == 1:
                        pf = [
                            (lambda eo=eo, half=half:
                             emit_p_group(b + 1, nb["ps"], eo, half, nb["PT"]))
                            for eo in range(NEO) for half in range(2)]
                    elif hp == 2:
                        for eo in range(NEO):
                            pf.append(lambda eo=eo:
                                      emit_q_group(b + 1, nb["xs"], eo,
                                                   nb["QU"], nb["QV"]))
                            pf.append(lambda eo=eo:
                                      emit_k_group(b + 1, nb["xs"], eo,
                                                   nb["KT"]))
                    elif hp == 3:
                        pf = [(lambda st=st:
                               emit_v_group(b + 1, nb["xs"], st, nb["VSB"]))
                              for st in range(NEO)]
                if hp == 2:
                    fillq.extend(pf)      # qk must precede next batch's u
                    fillq.extend(uf)
                else:
                    fillq.extend(uf)
                    fillq.extend(pf)

                if k in sc_cache:
                    ETS0, ETS1 = sc_cache.pop(k)
                else:
                    VP = vp_ring.pop(k)
                    ETS0 = emit_scores(b, 2 * hp, VP, cb["QU"], cb["KT"])
                    ETS1 = emit_scores(b, 2 * hp + 1, VP, cb["QU"], cb["KT"])
                if pending_av is not None:
                    finish_pair(*pending_av, last=False)
                pending_av = (b, hp, ETS0, ETS1)
                if k == NPAIR - 2:
                    # pre-emit the last pair's scores on the idle pup ring so
                    # its exp chain overlaps this phase instead of walling the
                    # endgame on ACT.
                    b2, hp2 = divmod(k + 1, 4)
                    c2 = ctx[b2]
                    VP2 = vp_ring.pop(k + 1)
                    E0 = emit_scores(b2, 2 * hp2, VP2, c2["QU"], c2["KT"],
                                     pac_pool=pup, pac_tag="pu")
                    E1 = emit_scores(b2, 2 * hp2 + 1, VP2, c2["QU"], c2["KT"],
                                     pac_pool=pup, pac_tag="pu")
                    sc_cache[k + 1] = (E0, E1)
                flush()
            finish_pair(*pending_av, last=True)

    _split_multiwaits(nc, mybir)
    return nc


def _prep_inputs(x, pos_emb, Wq, bq, Wk, bk, Wv, bv, Wp,
                 pos_bias_u, pos_bias_v):
    import ml_dtypes
    BF = ml_dtypes.bfloat16
    xT = np.ascontiguousarray(
        np.asarray(x, np.float32).transpose(0, 2, 1)).astype(BF)
    peT = np.zeros((B, E, S2 + 1), BF)
    peT[:, :, 0:S2] = np.asarray(pos_emb, np.float32).transpose(0, 2, 1).astype(BF)
    wqT = np.ascontiguousarray(np.asarray(Wq, np.float32).T.astype(BF))
    wkT = np.ascontiguousarray(np.asarray(Wk, np.float32).T.astype(BF))
    wvT = np.ascontiguousarray(np.asarray(Wv, np.float32).T.astype(BF))
    wpT = np.ascontiguousarray(np.asarray(Wp, np.float32).T.astype(BF))
    bias_u = (np.asarray(bq, np.float32)
              + np.asarray(pos_bias_u, np.float32).reshape(E))
    bias_v = (np.asarray(bq, np.float32)
              + np.asarray(pos_bias_v, np.float32).reshape(E))
    bu_p = np.ascontiguousarray(bias_u.reshape(4, 128).T)
    bdv_p = np.ascontiguousarray((bias_v - bias_u).reshape(4, 128).T)
    bk_p = np.ascontiguousarray(np.asarray(bk, np.float32).reshape(4, 128).T)
    common = {
        "wqT": wqT, "wkT": wkT, "wvT": wvT, "wpT": wpT,
        "bu": bu_p, "bdv": bdv_p, "bkk": bk_p,
        "bvp": np.ascontiguousarray(bias_v.reshape(4, 128).T),
        "bvec": np.asarray(bv, np.float32),
    }
    in_maps = []
    for c in range(N_CORES):
        m = dict(common)
        m["xT"] = xT[c * BPC:(c + 1) * BPC]
        m["peT"] = peT[c * BPC:(c + 1) * BPC]
        in_maps.append(m)
    return in_maps


def kernel(x, pos_emb, Wq, bq, Wk, bk, Wv, bv, Wp,
           pos_bias_u, pos_bias_v, legacy=0, **_):
    from concourse.bass_utils import run_bass_kernel_spmd

    if "nc" not in _CACHE:
        _CACHE["nc"] = _build_nc()
    nc = _CACHE["nc"]
    in_maps = _prep_inputs(x, pos_emb, Wq, bq, Wk, bk, Wv, bv, Wp,
                           pos_bias_u, pos_bias_v)
    res = run_bass_kernel_spmd(nc, in_maps, list(range(N_CORES))).results
    return np.concatenate([r["out"] for r in res], axis=0)   # [B, T, E]


# revision 7
# speedup vs baseline: 1.2832x; 1.0168x over previous
"""Trainium2 Bass kernel for nn_AttentionForONNX (Transformer-XL style
relative-position attention), v2.

Pipeline redesign over v1 (133.5us -> 105.8us in the TimelineSim cost
model):
  - PE pstate warmup: dummy matmuls on a scratch tile burn the 0.65/1.2
    GHz ramp while the first DMAs stream, so real matmuls run at 2.4 GHz.
  - x / Wq / Wk / Wv loaded in bf16 (output error is dominated by the
    score-path bf16 quantization; measured no change) - halves load DMA.
  - Chunked startup loads interleaved per-ei, and the batch-0 p projection
    runs ei-major so each arriving chunk is consumed immediately.
  - Global filler queue: projection eo-groups of the next batch and the
    u-band matmuls of pair k+2 are drained one group at a time into fill
    points inside the score/av phases, keeping the in-order PE busy while
    ACT/DVE drain PSUM.
  - 2-phase u-band lookahead (ubp bufs=3) so each pair's rel-shift DMA is
    issued a full phase early and never gates the transposes.
  - One merged diagonal shift DMA per pair (4x fewer HWDGE slots).
  - av/normalize stage software-pipelined one phase behind its scores so
    the ACT exp chain never gates the av matmuls; the last pair's scores
    are pre-emitted on the then-idle u-band PSUM ring for the same reason.
  - PSUM: u-band pool (2x2 banks) + shared 1-bank proj/av ring (2) +
    pac ring (2) = 8 banks.
  - Tail: per-head epilogue, avc on ACT, norms on DVE, per-head DMAs.
"""
import sys
import os

for _p in ("/opt/trn_rl_repo", "/root/.axon_site/_ro/trn_rl_repo"):
    if os.path.isdir(_p) and _p not in sys.path:
        sys.path.insert(0, _p)

import numpy as np

B, T, E, H = 16, 512, 512, 8
HD = E // H
S2 = 2 * T - 1
N_CORES = 8
BPC = B // N_CORES          # batches per core
SCALE = 1.0 / float(np.sqrt(HD))
N_WARM = 10

_CACHE = {}


def _split_multiwaits(nc, mybir):
    """walrus supports only one sync-wait per instruction: split extras
    into single-wait NOPs preceding the instruction."""
    n = 0
    for bb in nc.main_func.blocks:
        new_insts = []
        for ins in bb.instructions:
            si = ins.sync_info
            if si and si.on_wait and len(si.on_wait) > 1:
                waits = list(si.on_wait)
                for w in waits[:-1]:
                    nop = mybir.InstNoOp(name=f"{ins.name}-w{n}", ins=[], outs=[])
                    nop.engine = ins.engine
                    nop.sync_info = mybir.SyncInfo(on_wait=[w], on_update=[])
                    nc.register_instruction(nop, overwrite=True)
                    new_insts.append(nop)
                    n += 1
                ins.sync_info = mybir.SyncInfo(on_wait=[waits[-1]],
                                               on_update=list(si.on_update))
            new_insts.append(ins)
        bb.instructions[:] = new_insts
    return n


def _build_nc(bpc=BPC, n_warm=N_WARM, epp_bufs=4, ubpp_bufs=3, am0=0, am1=6, qu_act0=0, qu_act1=0):
    import concourse.bass as bass
    import concourse.mybir as mybir
    import concourse.tile as tile
    from concourse.ap import AP
    from concourse.masks import make_identity

    F32 = mybir.dt.float32
    F32R = mybir.dt.float32r
    BF16 = mybir.dt.bfloat16
    AT = mybir.AluOpType
    AF = mybir.ActivationFunctionType

    nc = bass.Bass("TRN2", target_bir_lowering=False)

    xT = nc.dram_tensor("xT", [bpc, E, T], BF16, kind="ExternalInput")
    peT = nc.dram_tensor("peT", [bpc, E, S2 + 1], BF16, kind="ExternalInput")
    wqT = nc.dram_tensor("wqT", [E, E], BF16, kind="ExternalInput")
    wkT = nc.dram_tensor("wkT", [E, E], BF16, kind="ExternalInput")
    wvT = nc.dram_tensor("wvT", [E, E], BF16, kind="ExternalInput")
    wpT = nc.dram_tensor("wpT", [E, E], BF16, kind="ExternalInput")
    # bias_u / (bias_v - bias_u) / bk packed [128, 4]: col eo = bias[eo*128:+128]
    bu = nc.dram_tensor("bu", [128, 4], F32, kind="ExternalInput")
    bdv = nc.dram_tensor("bdv", [128, 4], F32, kind="ExternalInput")
    bvp = nc.dram_tensor("bvp", [128, 4], F32, kind="ExternalInput")
    bkk = nc.dram_tensor("bkk", [128, 4], F32, kind="ExternalInput")
    bvec = nc.dram_tensor("bvec", [E], F32, kind="ExternalInput")   # bv for v
    out_d = nc.dram_tensor("out", [bpc, T, E], F32, kind="ExternalOutput")

    NEO = E // 128
    J0 = [384 - 128 * tt for tt in range(4)]
    W65 = 65 * H     # 520
    NPAIR = 4 * bpc

    with tile.TileContext(nc) as tc:
        with (
            tc.tile_pool(name="const", bufs=1) as const,
            tc.tile_pool(name="batch", bufs=1) as batch,
            tc.tile_pool(name="blate", bufs=2) as blate,
            tc.tile_pool(name="ubpp", bufs=ubpp_bufs) as ubpp,
            tc.tile_pool(name="vpp", bufs=2) as vpp,
            tc.tile_pool(name="epp", bufs=epp_bufs) as epp,
            tc.tile_pool(name="osb", bufs=2) as osb,
            tc.tile_pool(name="work", bufs=2) as work,
            tc.tile_pool(name="pup", bufs=2, space="PSUM") as pup,    # 4 banks
            tc.tile_pool(name="pjp", bufs=2, space="PSUM") as pjp,    # 2 banks
            tc.tile_pool(name="pacs", bufs=2, space="PSUM") as pacs,  # 2 banks
        ):
            # ---- warmup: matmuls on an uninitialized scratch tile burn the
            # PE pstate ramp while the first loads stream in. The psum
            # result is never read, so the garbage input is harmless, and
            # skipping the memset lets the PE start ~70ns in.
            WARM = const.tile([128, 257], BF16, tag="warm")
            nc.vector.memset(WARM[:, 256:257], 0.0)
            IDENT = const.tile([128, 128], BF16, tag="ident")
            make_identity(nc, IDENT[:])
            wps = pjp.tile([128, 512], F32, tag="pj", name="warmps")
            for _ in range(n_warm):
                nc.tensor.matmul(wps[:, 0:256], WARM[:, 0:128], WARM[:, 0:256],
                                 start=True, stop=True, skip_group_check=True)

            # ---- startup loads, chunked + interleaved: p path first.
            WP = const.tile([128, 4 * E], BF16, tag="wp")
            PET0 = batch.tile([128, 4 * (S2 + 1)], BF16, tag="pe0", name="peT0")
            XT0 = batch.tile([128, 4 * T], BF16, tag="xt0", name="xT0")
            WQ = const.tile([128, 4 * E], BF16, tag="wq")
            BU = const.tile([128, 4], F32, tag="bu")
            for ei in range(NEO):
                nc.sync.dma_start(
                    out=WP[:, 512 * ei:512 * (ei + 1)],
                    in_=AP(wpT, ei * 65536, [[512, 128], [1, 512]]))
                nc.sync.dma_start(
                    out=PET0[:, 1024 * ei:1024 * (ei + 1)],
                    in_=AP(peT, ei * 131072, [[1024, 128], [1, 1024]]))

            WK = const.tile([128, 4 * E], BF16, tag="wk")
            for c in range(2):
                nc.sync.dma_start(
                    out=XT0[:, 1024 * c:1024 * (c + 1)],
                    in_=AP(xT, c * 131072, [[512, 128], [65536, 2], [1, 512]]))
                nc.sync.dma_start(
                    out=WQ[:, 1024 * c:1024 * (c + 1)],
                    in_=AP(wqT, c * 131072, [[512, 128], [65536, 2], [1, 512]]))
                nc.sync.dma_start(
                    out=WK[:, 1024 * c:1024 * (c + 1)],
                    in_=AP(wkT, c * 131072, [[512, 128], [65536, 2], [1, 512]]))
            nc.sync.dma_start(out=BU, in_=bu[:])
            BDV = const.tile([128, 4], F32, tag="bdv")
            nc.sync.dma_start(out=BDV, in_=bdv[:])
            BK = const.tile([128, 4], F32, tag="bkk")
            nc.sync.dma_start(out=BK, in_=bkk[:])
            BVB = const.tile([128, E], F32, tag="bvb")
            nc.sync.dma_start(out=BVB, in_=AP(bvec, 0, [[0, 128], [1, E]]))
            WV = const.tile([128, 4 * E], BF16, tag="wv")
            nc.sync.dma_start(out=WV,
                              in_=AP(wvT, 0, [[512, 128], [65536, 4], [1, 512]]))

            def xs_of(XTb):
                return [XTb[:, 512 * ei:512 * (ei + 1)] for ei in range(NEO)]

            def ps_of(PETb):
                return [PETb[:, 1024 * ei:1024 * (ei + 1)] for ei in range(NEO)]

            def emit_loads(b):
                XTb = batch.tile([128, 4 * T], BF16, tag=f"xt{b}",
                                 name=f"xT{b}")
                PETb = batch.tile([128, 4 * (S2 + 1)], BF16, tag=f"pe{b}",
                                  name=f"peT{b}")
                for c in range(2):
                    nc.sync.dma_start(
                        out=XTb[:, 1024 * c:1024 * (c + 1)],
                        in_=AP(xT, b * 262144 + c * 131072,
                               [[512, 128], [65536, 2], [1, 512]]))
                for c in range(2):
                    nc.sync.dma_start(
                        out=PETb[:, 2048 * c:2048 * (c + 1)],
                        in_=AP(peT, b * 524288 + c * 262144,
                               [[1024, 128], [131072, 2], [1, 1024]]))
                return xs_of(XTb), ps_of(PETb)

            # ---- per-group projection emitters (each is one filler unit)
            def emit_p_group(b, PETs, eo, half, PT, on_act=False):
                pp = pjp.tile([128, 512], F32, tag="pj", name="pp")
                c0 = 512 * half
                for ei in range(NEO):
                    nc.tensor.matmul(
                        pp[:, 0:512],
                        WP[:, 512 * ei + 128 * eo:512 * ei + 128 * (eo + 1)],
                        PETs[ei][:, c0:c0 + 512],
                        start=(ei == 0), stop=(ei == NEO - 1))
                if PT[eo] is None:
                    PT[eo] = blate.tile([128, S2 + 1], BF16, tag=f"pt{eo}",
                                        name=f"pt{b}_{eo}")
                if on_act:
                    nc.scalar.activation(PT[eo][:, c0:c0 + 512],
                                         pp[:, 0:512], AF.Copy)
                else:
                    nc.vector.tensor_copy(PT[eo][:, c0:c0 + 512], pp[:, 0:512])

            def emit_q_group(b, XTs, eo, QU, QV, qv_act=False):
                qu_act = qu_act0 if b == 0 else qu_act1
                pq = pjp.tile([128, 512], F32, tag="pj", name="pq")
                for ei in range(NEO):
                    nc.tensor.matmul(
                        pq[:, 0:512],
                        WQ[:, 512 * ei + 128 * eo:512 * ei + 128 * (eo + 1)],
                        XTs[ei], start=(ei == 0), stop=(ei == NEO - 1))
                qu = blate.tile([128, T], F32R, tag=f"qu{eo}", name=f"qu{eo}")
                if qu_act:
                    nc.scalar.activation(qu[:], pq[:, 0:512], AF.Identity,
                                         bias=BU[:, eo:eo + 1])
                else:
                    nc.vector.tensor_scalar_add(qu[:], pq[:, 0:512],
                                                BU[:, eo:eo + 1])
                qv = blate.tile([128, T], BF16, tag=f"qv{eo}", name=f"qv{eo}")
                nc.gpsimd.tensor_scalar_add(qv[:], qu[:],
                                            BDV[:, eo:eo + 1])
                QU[eo] = qu
                QV[eo] = qv

            def emit_k_group(b, XTs, eo, KT):
                pk = pjp.tile([128, 512], F32, tag="pj", name="pk")
                for ei in range(NEO):
                    nc.tensor.matmul(
                        pk[:, 0:512],
                        WK[:, 512 * ei + 128 * eo:512 * ei + 128 * (eo + 1)],
                        XTs[ei], start=(ei == 0), stop=(ei == NEO - 1))
                kt = blate.tile([128, T], F32R, tag=f"kt{eo}", name=f"kt{eo}")
                nc.scalar.activation(kt[:], pk[:, 0:512], AF.Identity,
                                     bias=BK[:, eo:eo + 1])
                KT[eo] = kt

            def emit_v_group(b, XTs, st, VSB):
                pv = pjp.tile([128, 512], F32, tag="pj", name="pv")
                for ei in range(NEO):
                    nc.tensor.matmul(
                        pv[:, 0:E], XTs[ei][:, st * 128:(st + 1) * 128],
                        WV[:, 512 * ei:512 * (ei + 1)],
                        start=(ei == 0), stop=(ei == NEO - 1))
                vsb = blate.tile([128, W65], BF16, tag=f"v{st}", name=f"v{st}")
                nc.vector.tensor_tensor(
                    AP(vsb.tensor, 0, [[W65, 128], [65, H], [1, HD]]),
                    pv[:, 0:E], BVB[:], AT.add)
                nc.gpsimd.memset(
                    AP(vsb.tensor, HD, [[W65, 128], [65, H]]), 1.0)
                VSB[st] = vsb

            def emit_u_tt(b, h, tt, QV, PT, UBP, act_units=(0, 6)):
                """u band for (head, t_tile): one 2-bank psum tile, one copy."""
                hp, i = h // 2, h % 2
                r0 = 64 * i
                j0 = J0[tt]
                lqv = QV[hp][r0:r0 + 64, 128 * tt:128 * (tt + 1)]
                ua = pup.tile([128, 640], F32, tag="pu", name="ua")
                nc.tensor.matmul(ua[:, 0:512], lqv,
                                 PT[hp][r0:r0 + 64, j0:j0 + 512],
                                 start=True, stop=True,
                                 tile_position=(r0, 0))
                nc.tensor.matmul(ua[:, 512:640], lqv,
                                 PT[hp][r0:r0 + 64, j0 + 512:j0 + 640],
                                 start=True, stop=True,
                                 tile_position=(r0, 0))
                base = 1280 * tt + 640 * i
                if ((tt << 1) | i) in act_units:
                    nc.scalar.activation(UBP[:, base:base + 639],
                                         ua[:, 0:639], AF.Copy)
                else:
                    nc.vector.tensor_copy(UBP[:, base:base + 639],
                                          ua[:, 0:639])

            def alloc_ubp(k):
                return ubpp.tile([128, 4 * 1280], BF16, tag="ub",
                                 name=f"ub{k}")

            def emit_shift(k, UBP, split=False):
                """diagonal rel-shift DMA; split per-tt for the startup pairs
                whose shift is on the critical path."""
                vp = vpp.tile([128, 4096], BF16, tag="vp", name=f"vp{k}")
                if split:
                    for tt in range(4):
                        nc.sync.dma_start(
                            out=vp[:, 1024 * tt:1024 * (tt + 1)],
                            in_=AP(UBP.tensor, 127 + 1280 * tt,
                                   [[5119, 128], [640, 2], [1, 512]]))
                else:
                    nc.sync.dma_start(
                        out=vp,
                        in_=AP(UBP.tensor, 127,
                               [[5119, 128], [1280, 4], [640, 2], [1, 512]]))
                return vp

            fillq = []

            def fill():
                if fillq:
                    fillq.pop(0)()

            def flush():
                while fillq:
                    fillq.pop(0)()

            def emit_scores(b, h, VP, QU, KT, pac_pool=None, pac_tag="pac"):
                hp, r0, i = h // 2, 64 * (h % 2), h % 2
                pool = pac_pool if pac_pool is not None else pacs
                PAC = [None] * 4
                ETS = [None] * 4

                def emit_ac(j):
                    pac = pool.tile([128, T], F32, tag=pac_tag, name="pac")
                    nc.tensor.matmul(pac[:],
                                     KT[hp][r0:r0 + 64, 128 * j:128 * (j + 1)],
                                     QU[hp][r0:r0 + 64, :],
                                     start=True, stop=False,
                                     tile_position=(r0, 0),
                                     skip_group_check=True)
                    PAC[j] = pac

                def emit_texp(j):
                    pac = PAC[j]
                    for tt in range(4):
                        nc.tensor.matmul(
                            pac[:, 128 * tt:128 * (tt + 1)],
                            VP[:, 1024 * tt + 512 * i + 128 * j:
                                  1024 * tt + 512 * i + 128 * (j + 1)],
                            IDENT[:],
                            start=False, stop=(tt == 3),
                            skip_group_check=True)
                    ets = epp.tile([128, T], BF16, tag=f"e{j}", name=f"e{h}_{j}")
                    nc.scalar.activation(ets[:], pac[:], AF.Exp,
                                         bias=0.0, scale=SCALE)
                    ETS[j] = ets

                emit_ac(0)
                emit_ac(1)
                emit_texp(0)
                fill()
                emit_ac(2)
                emit_texp(1)
                fill()
                emit_ac(3)
                emit_texp(2)
                fill()
                emit_texp(3)
                fill()
                return ETS

            def emit_av(b, h, ETS, VSB, avc_act=None):
                av = pjp.tile([128, 4 * 65], F32, tag="pj", name=f"av{h}")
                for tt in range(4):
                    for j in range(4):
                        nc.tensor.matmul(
                            av[:, 65 * tt:65 * (tt + 1)],
                            ETS[j][:, 128 * tt:128 * (tt + 1)],
                            VSB[j][:, 65 * h:65 * (h + 1)],
                            start=(j == 0), stop=(j == 3))
                    fill()
                avc = work.tile([128, 4 * 65], F32, tag="avc", name=f"avc{h}")
                if avc_act is None:
                    avc_act = (h % 2 == 0)
                if avc_act:
                    nc.scalar.activation(avc[:], av[:], AF.Copy)
                else:
                    nc.vector.tensor_copy(avc[:], av[:])
                rec = work.tile([128, 4], F32, tag=f"rec{h}", name=f"rec{h}")
                nc.vector.reciprocal(
                    rec[:], AP(avc.tensor, HD, [[4 * 65, 128], [65, 4]]))
                return avc, rec

            def emit_norm(b, h, avc, rec, OQ, engines=None):
                c0 = 64 * (h % 2)
                for tt in range(4):
                    dst = OQ[:, 128 * tt + c0:128 * tt + c0 + 64]
                    src = avc[:, 65 * tt:65 * tt + 64]
                    if engines and engines[tt] == "dve":
                        nc.vector.tensor_scalar_mul(dst, src, rec[:, tt:tt + 1])
                    else:
                        nc.gpsimd.tensor_scalar_mul(dst, src, rec[:, tt:tt + 1])

            # ================= schedule =================
            # per-batch tile contexts; pair k = (b, hp) = divmod(k, 4)
            ctx = {0: {"QU": [None] * 4, "QV": [None] * 4, "KT": [None] * 4,
                       "PT": [None] * 4, "VSB": [None] * 4,
                       "xs": xs_of(XT0), "ps": ps_of(PET0)}}
            ubp_ring = {}
            vp_ring = {}

            def u_fillers(k):
                """closures for pair k's 8 u_tt groups (order tt-major).
                ACT/DVE copy split tuned per phase load: prologue pairs
                lean on ACT (idle there), late pairs stay 2/8."""
                b, hp = divmod(k, 4)
                ubp_ring[k] = alloc_ubp(k)
                c = ctx[b]
                act_units = (0, 2, 4, 6) if k < 2 else (tuple(u for u in (am0, am1) if u >= 0) if k < 6 else ((am0,) if am1 == 6 else ()))
                res = []
                for tt in range(4):
                    for i in range(2):
                        res.append(lambda tt=tt, i=i, b=b, hp=hp:
                                   emit_u_tt(b, 2 * hp + i, tt,
                                             c["QV"], c["PT"], ubp_ring[k],
                                             act_units))
                return res

            # ---- prologue: batch 0 projections with pair-0/1 u interleave
            # batch-0 p projection, ei-major: each arriving PET/WP chunk is
            # consumed immediately across all four eo tiles (pj ring is 4 deep)
            c0_ = ctx[0]
            for half in range(2):
                c0h = 512 * half
                pph = [(pup if eo < 2 else pjp).tile(
                    [128, 640 if eo < 2 else 512], F32,
                    tag="pu" if eo < 2 else "pj", name=f"pp{eo}")
                       for eo in range(NEO)]
                for ei in range(NEO):
                    for eo in range(NEO):
                        nc.tensor.matmul(
                            pph[eo][:, 0:512],
                            WP[:, 512 * ei + 128 * eo:512 * ei + 128 * (eo + 1)],
                            c0_["ps"][ei][:, c0h:c0h + 512],
                            start=(ei == 0), stop=(ei == NEO - 1),
                            skip_group_check=True)
                for eo in range(NEO):
                    if c0_["PT"][eo] is None:
                        c0_["PT"][eo] = blate.tile([128, S2 + 1], BF16,
                                                   tag=f"pt{eo}",
                                                   name=f"pt0_{eo}")
                    if half == 1:
                        nc.scalar.activation(
                            c0_["PT"][eo][:, c0h:c0h + 512],
                            pph[eo][:, 0:512], AF.Copy)
                    else:
                        nc.vector.tensor_copy(
                            c0_["PT"][eo][:, c0h:c0h + 512], pph[eo][:, 0:512])
            # pair-0 u fillers wait on the qu->qv chain; emit the first q/k
            # groups before draining any so the chain has latency cover.
            emit_q_group(0, c0_["xs"], 0, c0_["QU"], c0_["QV"])
            emit_k_group(0, c0_["xs"], 0, c0_["KT"])
            fillq.extend(u_fillers(0))
            for eo in range(1, NEO):
                emit_q_group(0, c0_["xs"], eo, c0_["QU"], c0_["QV"])
                fill()
                fill()
                emit_k_group(0, c0_["xs"], eo, c0_["KT"])
                fill()
            flush()
            vp_ring[0] = emit_shift(0, ubp_ring[0])
            fillq.extend(u_fillers(1))
            for st in range(NEO):
                emit_v_group(0, c0_["xs"], st, c0_["VSB"])
                fill()
                fill()
            flush()

            def finish_pair(b, hp, ETS0, ETS1, last):
                """av + normalize + output DMA for pair (b, hp) — emitted one
                phase late so the exp chain never gates the av matmuls."""
                cb = ctx[b]
                OQ = osb.tile([128, 512], F32, tag="oq", name=f"o{b}_{hp}")
                late = 4 * b + hp >= 5
                if not last:
                    avc0, rec0 = emit_av(b, 2 * hp, ETS0, cb["VSB"],
                                         avc_act=False if late else None)
                    avc1, rec1 = emit_av(b, 2 * hp + 1, ETS1, cb["VSB"],
                                         avc_act=False if late else None)
                    emit_norm(b, 2 * hp, avc0, rec0, OQ)
                    emit_norm(b, 2 * hp + 1, avc1, rec1, OQ)
                    nc.sync.dma_start(
                        out=AP(out_d, b * T * E + 128 * hp,
                               [[512, 128], [65536, 4], [1, 128]]),
                        in_=OQ[:])
                else:
                    # tail: per-head epilogue, avc on ACT, norms on DVE,
                    # split per-head DMAs
                    avc0, rec0 = emit_av(b, 2 * hp, ETS0, cb["VSB"],
                                         avc_act=True)
                    emit_norm(b, 2 * hp, avc0, rec0, OQ,
                              engines=("dve", "dve", "dve", "dve"))
                    nc.sync.dma_start(
                        out=AP(out_d, b * T * E + 128 * hp,
                               [[512, 128], [65536, 4], [1, 64]]),
                        in_=AP(OQ.tensor, 0, [[512, 128], [128, 4], [1, 64]]))
                    avc1, rec1 = emit_av(b, 2 * hp + 1, ETS1, cb["VSB"],
                                         avc_act=True)
                    emit_norm(b, 2 * hp + 1, avc1, rec1, OQ,
                              engines=("dve", "dve", "dve", "dve"))
                    nc.sync.dma_start(
                        out=AP(out_d, b * T * E + 128 * hp + 64,
                               [[512, 128], [65536, 4], [1, 64]]),
                        in_=AP(OQ.tensor, 64, [[512, 128], [128, 4], [1, 64]]))

            # ---- main pair loop (av stage pipelined one phase behind)
            pending_av = None
            sc_cache = {}
            for k in range(NPAIR):
                b, hp = divmod(k, 4)
                last = (k == NPAIR - 1)
                cb = ctx[b]
                if k + 1 < NPAIR:
                    vp_ring[k + 1] = emit_shift(k + 1, ubp_ring[k + 1])
                if hp == 0 and b + 1 < bpc:
                    nb = {"QU": [None] * 4, "QV": [None] * 4,
                          "KT": [None] * 4, "PT": [None] * 4,
                          "VSB": [None] * 4}
                    nb["xs"], nb["ps"] = emit_loads(b + 1)
                    ctx[b + 1] = nb
                # fillers for this phase
                if k + 2 < NPAIR:
                    uf = u_fillers(k + 2)
                else:
                    uf = []
                pf = []
                if b + 1 < bpc:
                    nb = ctx[b + 1]
                    if hp == 1:
                        pf = [
                            (lambda eo=eo, half=half:
                             emit_p_group(b + 1, nb["ps"], eo, half, nb["PT"]))
                            for eo in range(NEO) for half in range(2)]
                    elif hp == 2:
                        for eo in range(NEO):
                            pf.append(lambda eo=eo:
                                      emit_q_group(b + 1, nb["xs"], eo,
                                                   nb["QU"], nb["QV"]))
                            pf.append(lambda eo=eo:
                                      emit_k_group(b + 1, nb["xs"], eo,
                                                   nb["KT"]))
                    elif hp == 3:
                        pf = [(lambda st=st:
                               emit_v_group(b + 1, nb["xs"], st, nb["VSB"]))
                              for st in range(NEO)]
                if hp == 2:
                    fillq.extend(pf)      # qk must precede next batch's u
                    fillq.extend(uf)
                else:
                    fillq.extend(uf)
                    fillq.extend(pf)

                if k in sc_cache:
                    ETS0, ETS1 = sc_cache.pop(k)
                else:
                    VP = vp_ring.pop(k)
                    ETS0 = emit_scores(b, 2 * hp, VP, cb["QU"], cb["KT"])
                    ETS1 = emit_scores(b, 2 * hp + 1, VP, cb["QU"], cb["KT"])
                flush()
                if k == NPAIR - 2:
                    pass
                if pending_av is not None and k != NPAIR - 2:
                    finish_pair(*pending_av, last=False)
                if k == NPAIR - 2:
                    # pre-emit the last pair's scores on the idle pup ring so
                    # its exp chain overlaps this phase instead of walling the
                    # endgame on ACT.
                    b2, hp2 = divmod(k + 1, 4)
                    c2 = ctx[b2]
                    VP2 = vp_ring.pop(k + 1)
                    E0 = emit_scores(b2, 2 * hp2, VP2, c2["QU"], c2["KT"],
                                     pac_pool=pup, pac_tag="pu")
                    E1 = emit_scores(b2, 2 * hp2 + 1, VP2, c2["QU"], c2["KT"],
                                     pac_pool=pup, pac_tag="pu")
                    sc_cache[k + 1] = (E0, E1)
                    finish_pair(*pending_av, last=False)
                pending_av = (b, hp, ETS0, ETS1)
                flush()
            finish_pair(*pending_av, last=True)

    _split_multiwaits(nc, mybir)
    return nc


def _prep_inputs(x, pos_emb, Wq, bq, Wk, bk, Wv, bv, Wp,
                 pos_bias_u, pos_bias_v):
    import ml_dtypes
    BF = ml_dtypes.bfloat16
    xT = np.ascontiguousarray(
        np.asarray(x, np.float32).transpose(0, 2, 1)).astype(BF)
    peT = np.zeros((B, E, S2 + 1), BF)
    peT[:, :, 0:S2] = np.asarray(pos_emb, np.float32).transpose(0, 2, 1).astype(BF)
    wqT = np.ascontiguousarray(np.asarray(Wq, np.float32).T.astype(BF))
    wkT = np.ascontiguousarray(np.asarray(Wk, np.float32).T.astype(BF))
    wvT = np.ascontiguousarray(np.asarray(Wv, np.float32).T.astype(BF))
    wpT = np.ascontiguousarray(np.asarray(Wp, np.float32).T.astype(BF))
    bias_u = (np.asarray(bq, np.float32)
              + np.asarray(pos_bias_u, np.float32).reshape(E))
    bias_v = (np.asarray(bq, np.float32)
              + np.asarray(pos_bias_v, np.float32).reshape(E))
    bu_p = np.ascontiguousarray(bias_u.reshape(4, 128).T)
    bdv_p = np.ascontiguousarray((bias_v - bias_u).reshape(4, 128).T)
    bk_p = np.ascontiguousarray(np.asarray(bk, np.float32).reshape(4, 128).T)
    common = {
        "wqT": wqT, "wkT": wkT, "wvT": wvT, "wpT": wpT,
        "bu": bu_p, "bdv": bdv_p, "bkk": bk_p,
        "bvp": np.ascontiguousarray(bias_v.reshape(4, 128).T),
        "bvec": np.asarray(bv, np.float32),
    }
    in_maps = []
    for c in range(N_CORES):
        m = dict(common)
        m["xT"] = xT[c * BPC:(c + 1) * BPC]
        m["peT"] = peT[c * BPC:(c + 1) * BPC]
        in_maps.append(m)
    return in_maps


def kernel(x, pos_emb, Wq, bq, Wk, bk, Wv, bv, Wp,
           pos_bias_u, pos_bias_v, legacy=0, **_):
    from concourse.bass_utils import run_bass_kernel_spmd

    if "nc" not in _CACHE:
        _CACHE["nc"] = _build_nc()
    nc = _CACHE["nc"]
    in_maps = _prep_inputs(x, pos_emb, Wq, bq, Wk, bk, Wv, bv, Wp,
                           pos_bias_u, pos_bias_v)
    res = run_bass_kernel_spmd(nc, in_maps, list(range(N_CORES))).results
    return np.concatenate([r["out"] for r in res], axis=0)   # [B, T, E]


# revision 8
# speedup vs baseline: 1.2878x; 1.0035x over previous
"""Trainium2 Bass kernel for nn_AttentionForONNX (Transformer-XL style
relative-position attention), v2.

Pipeline redesign over v1 (133.5us -> 105.8us in the TimelineSim cost
model):
  - PE pstate warmup: dummy matmuls on a scratch tile burn the 0.65/1.2
    GHz ramp while the first DMAs stream, so real matmuls run at 2.4 GHz.
  - x / Wq / Wk / Wv loaded in bf16 (output error is dominated by the
    score-path bf16 quantization; measured no change) - halves load DMA.
  - Chunked startup loads interleaved per-ei, and the batch-0 p projection
    runs ei-major so each arriving chunk is consumed immediately.
  - Global filler queue: projection eo-groups of the next batch and the
    u-band matmuls of pair k+2 are drained one group at a time into fill
    points inside the score/av phases, keeping the in-order PE busy while
    ACT/DVE drain PSUM.
  - 2-phase u-band lookahead (ubp bufs=3) so each pair's rel-shift DMA is
    issued a full phase early and never gates the transposes.
  - One merged diagonal shift DMA per pair (4x fewer HWDGE slots).
  - av/normalize stage software-pipelined one phase behind its scores so
    the ACT exp chain never gates the av matmuls; the last pair's scores
    are pre-emitted on the then-idle u-band PSUM ring for the same reason.
  - PSUM: u-band pool (2x2 banks) + shared 1-bank proj/av ring (2) +
    pac ring (2) = 8 banks.
  - Tail: per-head epilogue, avc on ACT, norms on DVE, per-head DMAs.
"""
import sys
import os

for _p in ("/opt/trn_rl_repo", "/root/.axon_site/_ro/trn_rl_repo"):
    if os.path.isdir(_p) and _p not in sys.path:
        sys.path.insert(0, _p)

import numpy as np

B, T, E, H = 16, 512, 512, 8
HD = E // H
S2 = 2 * T - 1
N_CORES = 8
BPC = B // N_CORES          # batches per core
SCALE = 1.0 / float(np.sqrt(HD))
N_WARM = 10

_CACHE = {}


def _split_multiwaits(nc, mybir):
    """walrus supports only one sync-wait per instruction: split extras
    into single-wait NOPs preceding the instruction."""
    n = 0
    for bb in nc.main_func.blocks:
        new_insts = []
        for ins in bb.instructions:
            si = ins.sync_info
            if si and si.on_wait and len(si.on_wait) > 1:
                waits = list(si.on_wait)
                for w in waits[:-1]:
                    nop = mybir.InstNoOp(name=f"{ins.name}-w{n}", ins=[], outs=[])
                    nop.engine = ins.engine
                    nop.sync_info = mybir.SyncInfo(on_wait=[w], on_update=[])
                    nc.register_instruction(nop, overwrite=True)
                    new_insts.append(nop)
                    n += 1
                ins.sync_info = mybir.SyncInfo(on_wait=[waits[-1]],
                                               on_update=list(si.on_update))
            new_insts.append(ins)
        bb.instructions[:] = new_insts
    return n


def _build_nc(bpc=BPC, n_warm=N_WARM, epp_bufs=4, ubpp_bufs=3, am0=0, am1=6, qu_act0=0, qu_act1=0):
    import concourse.bass as bass
    import concourse.mybir as mybir
    import concourse.tile as tile
    from concourse.ap import AP
    from concourse.masks import make_identity

    F32 = mybir.dt.float32
    F32R = mybir.dt.float32r
    BF16 = mybir.dt.bfloat16
    AT = mybir.AluOpType
    AF = mybir.ActivationFunctionType

    nc = bass.Bass("TRN2", target_bir_lowering=False)

    xT = nc.dram_tensor("xT", [bpc, E, T], BF16, kind="ExternalInput")
    peT = nc.dram_tensor("peT", [bpc, E, S2 + 1], BF16, kind="ExternalInput")
    wqT = nc.dram_tensor("wqT", [E, E], BF16, kind="ExternalInput")
    wkT = nc.dram_tensor("wkT", [E, E], BF16, kind="ExternalInput")
    wvT = nc.dram_tensor("wvT", [E, E], BF16, kind="ExternalInput")
    wpT = nc.dram_tensor("wpT", [E, E], BF16, kind="ExternalInput")
    # bias_u / (bias_v - bias_u) / bk packed [128, 4]: col eo = bias[eo*128:+128]
    bu = nc.dram_tensor("bu", [128, 4], F32, kind="ExternalInput")
    bdv = nc.dram_tensor("bdv", [128, 4], F32, kind="ExternalInput")
    bvp = nc.dram_tensor("bvp", [128, 4], F32, kind="ExternalInput")
    bkk = nc.dram_tensor("bkk", [128, 4], F32, kind="ExternalInput")
    bvec = nc.dram_tensor("bvec", [E], F32, kind="ExternalInput")   # bv for v
    out_d = nc.dram_tensor("out", [bpc, T, E], BF16, kind="ExternalOutput")

    NEO = E // 128
    J0 = [384 - 128 * tt for tt in range(4)]
    W65 = 65 * H     # 520
    NPAIR = 4 * bpc

    with tile.TileContext(nc) as tc:
        with (
            tc.tile_pool(name="const", bufs=1) as const,
            tc.tile_pool(name="batch", bufs=1) as batch,
            tc.tile_pool(name="blate", bufs=2) as blate,
            tc.tile_pool(name="ubpp", bufs=ubpp_bufs) as ubpp,
            tc.tile_pool(name="vpp", bufs=2) as vpp,
            tc.tile_pool(name="epp", bufs=epp_bufs) as epp,
            tc.tile_pool(name="osb", bufs=2) as osb,
            tc.tile_pool(name="work", bufs=2) as work,
            tc.tile_pool(name="pup", bufs=2, space="PSUM") as pup,    # 4 banks
            tc.tile_pool(name="pjp", bufs=2, space="PSUM") as pjp,    # 2 banks
            tc.tile_pool(name="pacs", bufs=2, space="PSUM") as pacs,  # 2 banks
        ):
            # ---- warmup: matmuls on an uninitialized scratch tile burn the
            # PE pstate ramp while the first loads stream in. The psum
            # result is never read, so the garbage input is harmless, and
            # skipping the memset lets the PE start ~70ns in.
            WARM = const.tile([128, 257], BF16, tag="warm")
            nc.vector.memset(WARM[:, 256:257], 0.0)
            IDENT = const.tile([128, 128], BF16, tag="ident")
            make_identity(nc, IDENT[:])
            wps = pjp.tile([128, 512], F32, tag="pj", name="warmps")
            for _ in range(n_warm):
                nc.tensor.matmul(wps[:, 0:256], WARM[:, 0:128], WARM[:, 0:256],
                                 start=True, stop=True, skip_group_check=True)

            # ---- startup loads, chunked + interleaved: p path first.
            WP = const.tile([128, 4 * E], BF16, tag="wp")
            PET0 = batch.tile([128, 4 * (S2 + 1)], BF16, tag="pe0", name="peT0")
            XT0 = batch.tile([128, 4 * T], BF16, tag="xt0", name="xT0")
            WQ = const.tile([128, 4 * E], BF16, tag="wq")
            BU = const.tile([128, 4], F32, tag="bu")
            for ei in range(NEO):
                nc.sync.dma_start(
                    out=WP[:, 512 * ei:512 * (ei + 1)],
                    in_=AP(wpT, ei * 65536, [[512, 128], [1, 512]]))
                nc.sync.dma_start(
                    out=PET0[:, 1024 * ei:1024 * (ei + 1)],
                    in_=AP(peT, ei * 131072, [[1024, 128], [1, 1024]]))

            WK = const.tile([128, 4 * E], BF16, tag="wk")
            for c in range(2):
                nc.sync.dma_start(
                    out=XT0[:, 1024 * c:1024 * (c + 1)],
                    in_=AP(xT, c * 131072, [[512, 128], [65536, 2], [1, 512]]))
                nc.sync.dma_start(
                    out=WQ[:, 1024 * c:1024 * (c + 1)],
                    in_=AP(wqT, c * 131072, [[512, 128], [65536, 2], [1, 512]]))
                nc.sync.dma_start(
                    out=WK[:, 1024 * c:1024 * (c + 1)],
                    in_=AP(wkT, c * 131072, [[512, 128], [65536, 2], [1, 512]]))
            nc.sync.dma_start(out=BU, in_=bu[:])
            BDV = const.tile([128, 4], F32, tag="bdv")
            nc.sync.dma_start(out=BDV, in_=bdv[:])
            BK = const.tile([128, 4], F32, tag="bkk")
            nc.sync.dma_start(out=BK, in_=bkk[:])
            BVB = const.tile([128, E], F32, tag="bvb")
            nc.sync.dma_start(out=BVB, in_=AP(bvec, 0, [[0, 128], [1, E]]))
            WV = const.tile([128, 4 * E], BF16, tag="wv")
            nc.sync.dma_start(out=WV,
                              in_=AP(wvT, 0, [[512, 128], [65536, 4], [1, 512]]))

            def xs_of(XTb):
                return [XTb[:, 512 * ei:512 * (ei + 1)] for ei in range(NEO)]

            def ps_of(PETb):
                return [PETb[:, 1024 * ei:1024 * (ei + 1)] for ei in range(NEO)]

            def emit_loads(b):
                XTb = batch.tile([128, 4 * T], BF16, tag=f"xt{b}",
                                 name=f"xT{b}")
                PETb = batch.tile([128, 4 * (S2 + 1)], BF16, tag=f"pe{b}",
                                  name=f"peT{b}")
                for c in range(2):
                    nc.sync.dma_start(
                        out=XTb[:, 1024 * c:1024 * (c + 1)],
                        in_=AP(xT, b * 262144 + c * 131072,
                               [[512, 128], [65536, 2], [1, 512]]))
                for c in range(2):
                    nc.sync.dma_start(
                        out=PETb[:, 2048 * c:2048 * (c + 1)],
                        in_=AP(peT, b * 524288 + c * 262144,
                               [[1024, 128], [131072, 2], [1, 1024]]))
                return xs_of(XTb), ps_of(PETb)

            # ---- per-group projection emitters (each is one filler unit)
            def emit_p_group(b, PETs, eo, half, PT, on_act=False):
                pp = pjp.tile([128, 512], F32, tag="pj", name="pp")
                c0 = 512 * half
                for ei in range(NEO):
                    nc.tensor.matmul(
                        pp[:, 0:512],
                        WP[:, 512 * ei + 128 * eo:512 * ei + 128 * (eo + 1)],
                        PETs[ei][:, c0:c0 + 512],
                        start=(ei == 0), stop=(ei == NEO - 1))
                if PT[eo] is None:
                    PT[eo] = blate.tile([128, S2 + 1], BF16, tag=f"pt{eo}",
                                        name=f"pt{b}_{eo}")
                if on_act:
                    nc.scalar.activation(PT[eo][:, c0:c0 + 512],
                                         pp[:, 0:512], AF.Copy)
                else:
                    nc.vector.tensor_copy(PT[eo][:, c0:c0 + 512], pp[:, 0:512])

            def emit_q_group(b, XTs, eo, QU, QV, qv_act=False):
                qu_act = qu_act0 if b == 0 else qu_act1
                pq = pjp.tile([128, 512], F32, tag="pj", name="pq")
                for ei in range(NEO):
                    nc.tensor.matmul(
                        pq[:, 0:512],
                        WQ[:, 512 * ei + 128 * eo:512 * ei + 128 * (eo + 1)],
                        XTs[ei], start=(ei == 0), stop=(ei == NEO - 1))
                qu = blate.tile([128, T], F32R, tag=f"qu{eo}", name=f"qu{eo}")
                if qu_act:
                    nc.scalar.activation(qu[:], pq[:, 0:512], AF.Identity,
                                         bias=BU[:, eo:eo + 1])
                else:
                    nc.vector.tensor_scalar_add(qu[:], pq[:, 0:512],
                                                BU[:, eo:eo + 1])
                qv = blate.tile([128, T], BF16, tag=f"qv{eo}", name=f"qv{eo}")
                nc.gpsimd.tensor_scalar_add(qv[:], qu[:],
                                            BDV[:, eo:eo + 1])
                QU[eo] = qu
                QV[eo] = qv

            def emit_k_group(b, XTs, eo, KT):
                pk = pjp.tile([128, 512], F32, tag="pj", name="pk")
                for ei in range(NEO):
                    nc.tensor.matmul(
                        pk[:, 0:512],
                        WK[:, 512 * ei + 128 * eo:512 * ei + 128 * (eo + 1)],
                        XTs[ei], start=(ei == 0), stop=(ei == NEO - 1))
                kt = blate.tile([128, T], F32R, tag=f"kt{eo}", name=f"kt{eo}")
                nc.scalar.activation(kt[:], pk[:, 0:512], AF.Identity,
                                     bias=BK[:, eo:eo + 1])
                KT[eo] = kt

            def emit_v_group(b, XTs, st, VSB):
                pv = pjp.tile([128, 512], F32, tag="pj", name="pv")
                for ei in range(NEO):
                    nc.tensor.matmul(
                        pv[:, 0:E], XTs[ei][:, st * 128:(st + 1) * 128],
                        WV[:, 512 * ei:512 * (ei + 1)],
                        start=(ei == 0), stop=(ei == NEO - 1))
                vsb = blate.tile([128, W65], BF16, tag=f"v{st}", name=f"v{st}")
                nc.vector.tensor_tensor(
                    AP(vsb.tensor, 0, [[W65, 128], [65, H], [1, HD]]),
                    pv[:, 0:E], BVB[:], AT.add)
                nc.gpsimd.memset(
                    AP(vsb.tensor, HD, [[W65, 128], [65, H]]), 1.0)
                VSB[st] = vsb

            def emit_u_tt(b, h, tt, QV, PT, UBP, act_units=(0, 6)):
                """u band for (head, t_tile): one 2-bank psum tile, one copy."""
                hp, i = h // 2, h % 2
                r0 = 64 * i
                j0 = J0[tt]
                lqv = QV[hp][r0:r0 + 64, 128 * tt:128 * (tt + 1)]
                ua = pup.tile([128, 640], F32, tag="pu", name="ua")
                nc.tensor.matmul(ua[:, 0:512], lqv,
                                 PT[hp][r0:r0 + 64, j0:j0 + 512],
                                 start=True, stop=True,
                                 tile_position=(r0, 0))
                nc.tensor.matmul(ua[:, 512:640], lqv,
                                 PT[hp][r0:r0 + 64, j0 + 512:j0 + 640],
                                 start=True, stop=True,
                                 tile_position=(r0, 0))
                base = 1280 * tt + 640 * i
                if ((tt << 1) | i) in act_units:
                    nc.scalar.activation(UBP[:, base:base + 639],
                                         ua[:, 0:639], AF.Copy)
                else:
                    nc.vector.tensor_copy(UBP[:, base:base + 639],
                                          ua[:, 0:639])

            def alloc_ubp(k):
                return ubpp.tile([128, 4 * 1280], BF16, tag="ub",
                                 name=f"ub{k}")

            def emit_shift(k, UBP, split=False):
                """diagonal rel-shift DMA; split per-tt for the startup pairs
                whose shift is on the critical path."""
                vp = vpp.tile([128, 4096], BF16, tag="vp", name=f"vp{k}")
                if split:
                    for tt in range(4):
                        nc.sync.dma_start(
                            out=vp[:, 1024 * tt:1024 * (tt + 1)],
                            in_=AP(UBP.tensor, 127 + 1280 * tt,
                                   [[5119, 128], [640, 2], [1, 512]]))
                else:
                    nc.sync.dma_start(
                        out=vp,
                        in_=AP(UBP.tensor, 127,
                               [[5119, 128], [1280, 4], [640, 2], [1, 512]]))
                return vp

            fillq = []

            def fill():
                if fillq:
                    fillq.pop(0)()

            def flush():
                while fillq:
                    fillq.pop(0)()

            def emit_scores(b, h, VP, QU, KT, pac_pool=None, pac_tag="pac"):
                hp, r0, i = h // 2, 64 * (h % 2), h % 2
                pool = pac_pool if pac_pool is not None else pacs
                PAC = [None] * 4
                ETS = [None] * 4

                def emit_ac(j):
                    pac = pool.tile([128, T], F32, tag=pac_tag, name="pac")
                    nc.tensor.matmul(pac[:],
                                     KT[hp][r0:r0 + 64, 128 * j:128 * (j + 1)],
                                     QU[hp][r0:r0 + 64, :],
                                     start=True, stop=False,
                                     tile_position=(r0, 0),
                                     skip_group_check=True)
                    PAC[j] = pac

                def emit_texp(j):
                    pac = PAC[j]
                    for tt in range(4):
                        nc.tensor.matmul(
                            pac[:, 128 * tt:128 * (tt + 1)],
                            VP[:, 1024 * tt + 512 * i + 128 * j:
                                  1024 * tt + 512 * i + 128 * (j + 1)],
                            IDENT[:],
                            start=False, stop=(tt == 3),
                            skip_group_check=True)
                    ets = epp.tile([128, T], BF16, tag=f"e{j}", name=f"e{h}_{j}")
                    nc.scalar.activation(ets[:], pac[:], AF.Exp,
                                         bias=0.0, scale=SCALE)
                    ETS[j] = ets

                emit_ac(0)
                emit_ac(1)
                emit_texp(0)
                fill()
                emit_ac(2)
                emit_texp(1)
                fill()
                emit_ac(3)
                emit_texp(2)
                fill()
                emit_texp(3)
                fill()
                return ETS

            def emit_av(b, h, ETS, VSB, avc_act=None):
                av = pjp.tile([128, 4 * 65], F32, tag="pj", name=f"av{h}")
                for tt in range(4):
                    for j in range(4):
                        nc.tensor.matmul(
                            av[:, 65 * tt:65 * (tt + 1)],
                            ETS[j][:, 128 * tt:128 * (tt + 1)],
                            VSB[j][:, 65 * h:65 * (h + 1)],
                            start=(j == 0), stop=(j == 3))
                    fill()
                avc = work.tile([128, 4 * 65], F32, tag="avc", name=f"avc{h}")
                if avc_act is None:
                    avc_act = (h % 2 == 0)
                if avc_act:
                    nc.scalar.activation(avc[:], av[:], AF.Copy)
                else:
                    nc.vector.tensor_copy(avc[:], av[:])
                rec = work.tile([128, 4], F32, tag=f"rec{h}", name=f"rec{h}")
                nc.vector.reciprocal(
                    rec[:], AP(avc.tensor, HD, [[4 * 65, 128], [65, 4]]))
                return avc, rec

            def emit_norm(b, h, avc, rec, OQ, engines=None):
                c0 = 64 * (h % 2)
                for tt in range(4):
                    dst = OQ[:, 128 * tt + c0:128 * tt + c0 + 64]
                    src = avc[:, 65 * tt:65 * tt + 64]
                    if engines and engines[tt] == "dve":
                        nc.vector.tensor_scalar_mul(dst, src, rec[:, tt:tt + 1])
                    else:
                        nc.gpsimd.tensor_scalar_mul(dst, src, rec[:, tt:tt + 1])

            # ================= schedule =================
            # per-batch tile contexts; pair k = (b, hp) = divmod(k, 4)
            ctx = {0: {"QU": [None] * 4, "QV": [None] * 4, "KT": [None] * 4,
                       "PT": [None] * 4, "VSB": [None] * 4,
                       "xs": xs_of(XT0), "ps": ps_of(PET0)}}
            ubp_ring = {}
            vp_ring = {}

            def u_fillers(k):
                """closures for pair k's 8 u_tt groups (order tt-major).
                ACT/DVE copy split tuned per phase load: prologue pairs
                lean on ACT (idle there), late pairs stay 2/8."""
                b, hp = divmod(k, 4)
                ubp_ring[k] = alloc_ubp(k)
                c = ctx[b]
                act_units = (0, 2, 4, 6) if k < 2 else (tuple(u for u in (am0, am1) if u >= 0) if k < 6 else ((am0,) if am1 == 6 else ()))
                res = []
                for tt in range(4):
                    for i in range(2):
                        res.append(lambda tt=tt, i=i, b=b, hp=hp:
                                   emit_u_tt(b, 2 * hp + i, tt,
                                             c["QV"], c["PT"], ubp_ring[k],
                                             act_units))
                return res

            # ---- prologue: batch 0 projections with pair-0/1 u interleave
            # batch-0 p projection, ei-major: each arriving PET/WP chunk is
            # consumed immediately across all four eo tiles (pj ring is 4 deep)
            c0_ = ctx[0]
            for half in range(2):
                c0h = 512 * half
                pph = [(pup if eo < 2 else pjp).tile(
                    [128, 640 if eo < 2 else 512], F32,
                    tag="pu" if eo < 2 else "pj", name=f"pp{eo}")
                       for eo in range(NEO)]
                for ei in range(NEO):
                    for eo in range(NEO):
                        nc.tensor.matmul(
                            pph[eo][:, 0:512],
                            WP[:, 512 * ei + 128 * eo:512 * ei + 128 * (eo + 1)],
                            c0_["ps"][ei][:, c0h:c0h + 512],
                            start=(ei == 0), stop=(ei == NEO - 1),
                            skip_group_check=True)
                for eo in range(NEO):
                    if c0_["PT"][eo] is None:
                        c0_["PT"][eo] = blate.tile([128, S2 + 1], BF16,
                                                   tag=f"pt{eo}",
                                                   name=f"pt0_{eo}")
                    if half == 1:
                        nc.scalar.activation(
                            c0_["PT"][eo][:, c0h:c0h + 512],
                            pph[eo][:, 0:512], AF.Copy)
                    else:
                        nc.vector.tensor_copy(
                            c0_["PT"][eo][:, c0h:c0h + 512], pph[eo][:, 0:512])
            # pair-0 u fillers wait on the qu->qv chain; emit the first q/k
            # groups before draining any so the chain has latency cover.
            emit_q_group(0, c0_["xs"], 0, c0_["QU"], c0_["QV"])
            emit_k_group(0, c0_["xs"], 0, c0_["KT"])
            fillq.extend(u_fillers(0))
            for eo in range(1, NEO):
                emit_q_group(0, c0_["xs"], eo, c0_["QU"], c0_["QV"])
                fill()
                fill()
                emit_k_group(0, c0_["xs"], eo, c0_["KT"])
                fill()
            flush()
            vp_ring[0] = emit_shift(0, ubp_ring[0])
            fillq.extend(u_fillers(1))
            for st in range(NEO):
                emit_v_group(0, c0_["xs"], st, c0_["VSB"])
                fill()
                fill()
            flush()

            def finish_pair(b, hp, ETS0, ETS1, last):
                """av + normalize + output DMA for pair (b, hp) — emitted one
                phase late so the exp chain never gates the av matmuls."""
                cb = ctx[b]
                OQ = osb.tile([128, 512], BF16, tag="oq", name=f"o{b}_{hp}")
                late = 4 * b + hp >= 5
                if not last:
                    avc0, rec0 = emit_av(b, 2 * hp, ETS0, cb["VSB"],
                                         avc_act=False if late else None)
                    avc1, rec1 = emit_av(b, 2 * hp + 1, ETS1, cb["VSB"],
                                         avc_act=False if late else None)
                    emit_norm(b, 2 * hp, avc0, rec0, OQ)
                    emit_norm(b, 2 * hp + 1, avc1, rec1, OQ)
                    nc.sync.dma_start(
                        out=AP(out_d, b * T * E + 128 * hp,
                               [[512, 128], [65536, 4], [1, 128]]),
                        in_=OQ[:])
                else:
                    # tail: per-head epilogue, avc on ACT, norms on DVE,
                    # split per-head DMAs
                    avc0, rec0 = emit_av(b, 2 * hp, ETS0, cb["VSB"],
                                         avc_act=True)
                    emit_norm(b, 2 * hp, avc0, rec0, OQ,
                              engines=("dve", "dve", "dve", "dve"))
                    nc.sync.dma_start(
                        out=AP(out_d, b * T * E + 128 * hp,
                               [[512, 128], [65536, 4], [1, 64]]),
                        in_=AP(OQ.tensor, 0, [[512, 128], [128, 4], [1, 64]]))
                    avc1, rec1 = emit_av(b, 2 * hp + 1, ETS1, cb["VSB"],
                                         avc_act=True)
                    emit_norm(b, 2 * hp + 1, avc1, rec1, OQ,
                              engines=("dve", "dve", "dve", "dve"))
                    nc.sync.dma_start(
                        out=AP(out_d, b * T * E + 128 * hp + 64,
                               [[512, 128], [65536, 4], [1, 64]]),
                        in_=AP(OQ.tensor, 64, [[512, 128], [128, 4], [1, 64]]))

            # ---- main pair loop (av stage pipelined one phase behind)
            pending_av = None
            sc_cache = {}
            for k in range(NPAIR):
                b, hp = divmod(k, 4)
                last = (k == NPAIR - 1)
                cb = ctx[b]
                if k + 1 < NPAIR:
                    vp_ring[k + 1] = emit_shift(k + 1, ubp_ring[k + 1])
                if hp == 0 and b + 1 < bpc:
                    nb = {"QU": [None] * 4, "QV": [None] * 4,
                          "KT": [None] * 4, "PT": [None] * 4,
                          "VSB": [None] * 4}
                    nb["xs"], nb["ps"] = emit_loads(b + 1)
                    ctx[b + 1] = nb
                # fillers for this phase
                if k + 2 < NPAIR:
                    uf = u_fillers(k + 2)
                else:
                    uf = []
                pf = []
                if b + 1 < bpc:
                    nb = ctx[b + 1]
                    if hp == 1:
                        pf = [
                            (lambda eo=eo, half=half:
                             emit_p_group(b + 1, nb["ps"], eo, half, nb["PT"]))
                            for eo in range(NEO) for half in range(2)]
                    elif hp == 2:
                        for eo in range(NEO):
                            pf.append(lambda eo=eo:
                                      emit_q_group(b + 1, nb["xs"], eo,
                                                   nb["QU"], nb["QV"]))
                            pf.append(lambda eo=eo:
                                      emit_k_group(b + 1, nb["xs"], eo,
                                                   nb["KT"]))
                    elif hp == 3:
                        pf = [(lambda st=st:
                               emit_v_group(b + 1, nb["xs"], st, nb["VSB"]))
                              for st in range(NEO)]
                if hp == 2:
                    fillq.extend(pf)      # qk must precede next batch's u
                    fillq.extend(uf)
                else:
                    fillq.extend(uf)
                    fillq.extend(pf)

                if k in sc_cache:
                    ETS0, ETS1 = sc_cache.pop(k)
                else:
                    VP = vp_ring.pop(k)
                    ETS0 = emit_scores(b, 2 * hp, VP, cb["QU"], cb["KT"])
                    ETS1 = emit_scores(b, 2 * hp + 1, VP, cb["QU"], cb["KT"])
                flush()
                if k == NPAIR - 2:
                    pass
                if pending_av is not None and k != NPAIR - 2:
                    finish_pair(*pending_av, last=False)
                if k == NPAIR - 2:
                    # pre-emit the last pair's scores on the idle pup ring so
                    # its exp chain overlaps this phase instead of walling the
                    # endgame on ACT.
                    b2, hp2 = divmod(k + 1, 4)
                    c2 = ctx[b2]
                    VP2 = vp_ring.pop(k + 1)
                    E0 = emit_scores(b2, 2 * hp2, VP2, c2["QU"], c2["KT"],
                                     pac_pool=pup, pac_tag="pu")
                    E1 = emit_scores(b2, 2 * hp2 + 1, VP2, c2["QU"], c2["KT"],
                                     pac_pool=pup, pac_tag="pu")
                    sc_cache[k + 1] = (E0, E1)
                    finish_pair(*pending_av, last=False)
                pending_av = (b, hp, ETS0, ETS1)
                flush()
            finish_pair(*pending_av, last=True)

    _split_multiwaits(nc, mybir)
    return nc


def _prep_inputs(x, pos_emb, Wq, bq, Wk, bk, Wv, bv, Wp,
                 pos_bias_u, pos_bias_v):
    import ml_dtypes
    BF = ml_dtypes.bfloat16
    xT = np.ascontiguousarray(
        np.asarray(x, np.float32).transpose(0, 2, 1)).astype(BF)
    peT = np.zeros((B, E, S2 + 1), BF)
    peT[:, :, 0:S2] = np.asarray(pos_emb, np.float32).transpose(0, 2, 1).astype(BF)
    wqT = np.ascontiguousarray(np.asarray(Wq, np.float32).T.astype(BF))
    wkT = np.ascontiguousarray(np.asarray(Wk, np.float32).T.astype(BF))
    wvT = np.ascontiguousarray(np.asarray(Wv, np.float32).T.astype(BF))
    wpT = np.ascontiguousarray(np.asarray(Wp, np.float32).T.astype(BF))
    bias_u = (np.asarray(bq, np.float32)
              + np.asarray(pos_bias_u, np.float32).reshape(E))
    bias_v = (np.asarray(bq, np.float32)
              + np.asarray(pos_bias_v, np.float32).reshape(E))
    bu_p = np.ascontiguousarray(bias_u.reshape(4, 128).T)
    bdv_p = np.ascontiguousarray((bias_v - bias_u).reshape(4, 128).T)
    bk_p = np.ascontiguousarray(np.asarray(bk, np.float32).reshape(4, 128).T)
    common = {
        "wqT": wqT, "wkT": wkT, "wvT": wvT, "wpT": wpT,
        "bu": bu_p, "bdv": bdv_p, "bkk": bk_p,
        "bvp": np.ascontiguousarray(bias_v.reshape(4, 128).T),
        "bvec": np.asarray(bv, np.float32),
    }
    in_maps = []
    for c in range(N_CORES):
        m = dict(common)
        m["xT"] = xT[c * BPC:(c + 1) * BPC]
        m["peT"] = peT[c * BPC:(c + 1) * BPC]
        in_maps.append(m)
    return in_maps


def kernel(x, pos_emb, Wq, bq, Wk, bk, Wv, bv, Wp,
           pos_bias_u, pos_bias_v, legacy=0, **_):
    from concourse.bass_utils import run_bass_kernel_spmd

    if "nc" not in _CACHE:
        _CACHE["nc"] = _build_nc()
    nc = _CACHE["nc"]
    in_maps = _prep_inputs(x, pos_emb, Wq, bq, Wk, bk, Wv, bv, Wp,
                           pos_bias_u, pos_bias_v)
    res = run_bass_kernel_spmd(nc, in_maps, list(range(N_CORES))).results
    return np.concatenate(
        [np.asarray(r["out"], np.float32) for r in res], axis=0)   # [B, T, E]


# revision 9
# speedup vs baseline: 1.2890x; 1.0010x over previous
"""Trainium2 Bass kernel for nn_AttentionForONNX (Transformer-XL style
relative-position attention), v2.

Pipeline redesign over v1 (133.5us -> 105.8us in the TimelineSim cost
model):
  - PE pstate warmup: dummy matmuls on a scratch tile burn the 0.65/1.2
    GHz ramp while the first DMAs stream, so real matmuls run at 2.4 GHz.
  - x / Wq / Wk / Wv loaded in bf16 (output error is dominated by the
    score-path bf16 quantization; measured no change) - halves load DMA.
  - Chunked startup loads interleaved per-ei, and the batch-0 p projection
    runs ei-major so each arriving chunk is consumed immediately.
  - Global filler queue: projection eo-groups of the next batch and the
    u-band matmuls of pair k+2 are drained one group at a time into fill
    points inside the score/av phases, keeping the in-order PE busy while
    ACT/DVE drain PSUM.
  - 2-phase u-band lookahead (ubp bufs=3) so each pair's rel-shift DMA is
    issued a full phase early and never gates the transposes.
  - One merged diagonal shift DMA per pair (4x fewer HWDGE slots).
  - av/normalize stage software-pipelined one phase behind its scores so
    the ACT exp chain never gates the av matmuls; the last pair's scores
    are pre-emitted on the then-idle u-band PSUM ring for the same reason.
  - PSUM: u-band pool (2x2 banks) + shared 1-bank proj/av ring (2) +
    pac ring (2) = 8 banks.
  - Tail: per-head epilogue, avc on ACT, norms on DVE, per-head DMAs.
"""
import sys
import os

for _p in ("/opt/trn_rl_repo", "/root/.axon_site/_ro/trn_rl_repo"):
    if os.path.isdir(_p) and _p not in sys.path:
        sys.path.insert(0, _p)

import numpy as np

B, T, E, H = 16, 512, 512, 8
HD = E // H
S2 = 2 * T - 1
N_CORES = 8
BPC = B // N_CORES          # batches per core
SCALE = 1.0 / float(np.sqrt(HD))
N_WARM = 10

_CACHE = {}


def _split_multiwaits(nc, mybir):
    """walrus supports only one sync-wait per instruction: split extras
    into single-wait NOPs preceding the instruction."""
    n = 0
    for bb in nc.main_func.blocks:
        new_insts = []
        for ins in bb.instructions:
            si = ins.sync_info
            if si and si.on_wait and len(si.on_wait) > 1:
                waits = list(si.on_wait)
                for w in waits[:-1]:
                    nop = mybir.InstNoOp(name=f"{ins.name}-w{n}", ins=[], outs=[])
                    nop.engine = ins.engine
                    nop.sync_info = mybir.SyncInfo(on_wait=[w], on_update=[])
                    nc.register_instruction(nop, overwrite=True)
                    new_insts.append(nop)
                    n += 1
                ins.sync_info = mybir.SyncInfo(on_wait=[waits[-1]],
                                               on_update=list(si.on_update))
            new_insts.append(ins)
        bb.instructions[:] = new_insts
    return n


def _build_nc(bpc=BPC, n_warm=N_WARM, epp_bufs=4, ubpp_bufs=3, am0=0, am1=6, qu_act0=0, qu_act1=0):
    import concourse.bass as bass
    import concourse.mybir as mybir
    import concourse.tile as tile
    from concourse.ap import AP
    from concourse.masks import make_identity

    F32 = mybir.dt.float32
    F32R = mybir.dt.float32r
    BF16 = mybir.dt.bfloat16
    AT = mybir.AluOpType
    AF = mybir.ActivationFunctionType

    nc = bass.Bass("TRN2", target_bir_lowering=False)

    xT = nc.dram_tensor("xT", [bpc, E, T], BF16, kind="ExternalInput")
    peT = nc.dram_tensor("peT", [bpc, E, S2 + 1], BF16, kind="ExternalInput")
    wqT = nc.dram_tensor("wqT", [E, E], BF16, kind="ExternalInput")
    wkT = nc.dram_tensor("wkT", [E, E], BF16, kind="ExternalInput")
    wvT = nc.dram_tensor("wvT", [E, E], BF16, kind="ExternalInput")
    wpT = nc.dram_tensor("wpT", [E, E], BF16, kind="ExternalInput")
    # bias_u / (bias_v - bias_u) / bk packed [128, 4]: col eo = bias[eo*128:+128]
    bu = nc.dram_tensor("bu", [128, 4], F32, kind="ExternalInput")
    bdv = nc.dram_tensor("bdv", [128, 4], F32, kind="ExternalInput")
    bvp = nc.dram_tensor("bvp", [128, 4], F32, kind="ExternalInput")
    bkk = nc.dram_tensor("bkk", [128, 4], F32, kind="ExternalInput")
    bvec = nc.dram_tensor("bvec", [E], F32, kind="ExternalInput")   # bv for v
    out_d = nc.dram_tensor("out", [bpc, T, E], BF16, kind="ExternalOutput")

    NEO = E // 128
    J0 = [384 - 128 * tt for tt in range(4)]
    W65 = 65 * H     # 520
    NPAIR = 4 * bpc

    with tile.TileContext(nc) as tc:
        with (
            tc.tile_pool(name="const", bufs=1) as const,
            tc.tile_pool(name="batch", bufs=1) as batch,
            tc.tile_pool(name="blate", bufs=2) as blate,
            tc.tile_pool(name="ubpp", bufs=ubpp_bufs) as ubpp,
            tc.tile_pool(name="vpp", bufs=2) as vpp,
            tc.tile_pool(name="epp", bufs=epp_bufs) as epp,
            tc.tile_pool(name="osb", bufs=2) as osb,
            tc.tile_pool(name="work", bufs=2) as work,
            tc.tile_pool(name="pup", bufs=2, space="PSUM") as pup,    # 4 banks
            tc.tile_pool(name="pjp", bufs=2, space="PSUM") as pjp,    # 2 banks
            tc.tile_pool(name="pacs", bufs=2, space="PSUM") as pacs,  # 2 banks
        ):
            # ---- warmup: matmuls on an uninitialized scratch tile burn the
            # PE pstate ramp while the first loads stream in. The psum
            # result is never read, so the garbage input is harmless, and
            # skipping the memset lets the PE start ~70ns in.
            WARM = const.tile([128, 257], BF16, tag="warm")
            nc.vector.memset(WARM[:, 256:257], 0.0)
            IDENT = const.tile([128, 128], BF16, tag="ident")
            make_identity(nc, IDENT[:])
            wps = pjp.tile([128, 512], F32, tag="pj", name="warmps")
            for _ in range(n_warm):
                nc.tensor.matmul(wps[:, 0:256], WARM[:, 0:128], WARM[:, 0:256],
                                 start=True, stop=True, skip_group_check=True)

            # ---- startup loads, chunked + interleaved: p path first.
            WP = const.tile([128, 4 * E], BF16, tag="wp")
            PET0 = batch.tile([128, 4 * (S2 + 1)], BF16, tag="pe0", name="peT0")
            XT0 = batch.tile([128, 4 * T], BF16, tag="xt0", name="xT0")
            WQ = const.tile([128, 4 * E], BF16, tag="wq")
            BU = const.tile([128, 4], F32, tag="bu")
            for ei in range(NEO):
                nc.sync.dma_start(
                    out=WP[:, 512 * ei:512 * (ei + 1)],
                    in_=AP(wpT, ei * 65536, [[512, 128], [1, 512]]))
                nc.sync.dma_start(
                    out=PET0[:, 1024 * ei:1024 * (ei + 1)],
                    in_=AP(peT, ei * 131072, [[1024, 128], [1, 1024]]))

            WK = const.tile([128, 4 * E], BF16, tag="wk")
            for c in range(2):
                nc.sync.dma_start(
                    out=XT0[:, 1024 * c:1024 * (c + 1)],
                    in_=AP(xT, c * 131072, [[512, 128], [65536, 2], [1, 512]]))
                nc.sync.dma_start(
                    out=WQ[:, 1024 * c:1024 * (c + 1)],
                    in_=AP(wqT, c * 131072, [[512, 128], [65536, 2], [1, 512]]))
                nc.sync.dma_start(
                    out=WK[:, 1024 * c:1024 * (c + 1)],
                    in_=AP(wkT, c * 131072, [[512, 128], [65536, 2], [1, 512]]))
            nc.sync.dma_start(out=BU, in_=bu[:])
            BDV = const.tile([128, 4], F32, tag="bdv")
            nc.sync.dma_start(out=BDV, in_=bdv[:])
            BK = const.tile([128, 4], F32, tag="bkk")
            nc.sync.dma_start(out=BK, in_=bkk[:])
            BVB = const.tile([128, E], F32, tag="bvb")
            nc.sync.dma_start(out=BVB, in_=AP(bvec, 0, [[0, 128], [1, E]]))
            WV = const.tile([128, 4 * E], BF16, tag="wv")
            nc.sync.dma_start(out=WV,
                              in_=AP(wvT, 0, [[512, 128], [65536, 4], [1, 512]]))

            def xs_of(XTb):
                return [XTb[:, 512 * ei:512 * (ei + 1)] for ei in range(NEO)]

            def ps_of(PETb):
                return [PETb[:, 1024 * ei:1024 * (ei + 1)] for ei in range(NEO)]

            def emit_loads(b):
                XTb = batch.tile([128, 4 * T], BF16, tag=f"xt{b}",
                                 name=f"xT{b}")
                PETb = batch.tile([128, 4 * (S2 + 1)], BF16, tag=f"pe{b}",
                                  name=f"peT{b}")
                for c in range(2):
                    nc.sync.dma_start(
                        out=XTb[:, 1024 * c:1024 * (c + 1)],
                        in_=AP(xT, b * 262144 + c * 131072,
                               [[512, 128], [65536, 2], [1, 512]]))
                for c in range(2):
                    nc.sync.dma_start(
                        out=PETb[:, 2048 * c:2048 * (c + 1)],
                        in_=AP(peT, b * 524288 + c * 262144,
                               [[1024, 128], [131072, 2], [1, 1024]]))
                return xs_of(XTb), ps_of(PETb)

            # ---- per-group projection emitters (each is one filler unit)
            def emit_p_group(b, PETs, eo, half, PT, on_act=False):
                pp = pjp.tile([128, 512], F32, tag="pj", name="pp")
                c0 = 512 * half
                for ei in range(NEO):
                    nc.tensor.matmul(
                        pp[:, 0:512],
                        WP[:, 512 * ei + 128 * eo:512 * ei + 128 * (eo + 1)],
                        PETs[ei][:, c0:c0 + 512],
                        start=(ei == 0), stop=(ei == NEO - 1))
                if PT[eo] is None:
                    PT[eo] = blate.tile([128, S2 + 1], BF16, tag=f"pt{eo}",
                                        name=f"pt{b}_{eo}")
                if on_act:
                    nc.scalar.activation(PT[eo][:, c0:c0 + 512],
                                         pp[:, 0:512], AF.Copy)
                else:
                    nc.vector.tensor_copy(PT[eo][:, c0:c0 + 512], pp[:, 0:512])

            def emit_q_group(b, XTs, eo, QU, QV, qv_act=False):
                qu_act = qu_act0 if b == 0 else qu_act1
                pq = pjp.tile([128, 512], F32, tag="pj", name="pq")
                for ei in range(NEO):
                    nc.tensor.matmul(
                        pq[:, 0:512],
                        WQ[:, 512 * ei + 128 * eo:512 * ei + 128 * (eo + 1)],
                        XTs[ei], start=(ei == 0), stop=(ei == NEO - 1))
                qu = blate.tile([128, T], F32R, tag=f"qu{eo}", name=f"qu{eo}")
                if qu_act:
                    nc.scalar.activation(qu[:], pq[:, 0:512], AF.Identity,
                                         bias=BU[:, eo:eo + 1])
                else:
                    nc.vector.tensor_scalar_add(qu[:], pq[:, 0:512],
                                                BU[:, eo:eo + 1])
                qv = blate.tile([128, T], BF16, tag=f"qv{eo}", name=f"qv{eo}")
                nc.gpsimd.tensor_scalar_add(qv[:], qu[:],
                                            BDV[:, eo:eo + 1])
                QU[eo] = qu
                QV[eo] = qv

            def emit_k_group(b, XTs, eo, KT):
                pk = pjp.tile([128, 512], F32, tag="pj", name="pk")
                for ei in range(NEO):
                    nc.tensor.matmul(
                        pk[:, 0:512],
                        WK[:, 512 * ei + 128 * eo:512 * ei + 128 * (eo + 1)],
                        XTs[ei], start=(ei == 0), stop=(ei == NEO - 1))
                kt = blate.tile([128, T], F32R, tag=f"kt{eo}", name=f"kt{eo}")
                nc.scalar.activation(kt[:], pk[:, 0:512], AF.Identity,
                                     bias=BK[:, eo:eo + 1])
                KT[eo] = kt

            def emit_v_group(b, XTs, st, VSB):
                pv = pjp.tile([128, 512], F32, tag="pj", name="pv")
                for ei in range(NEO):
                    nc.tensor.matmul(
                        pv[:, 0:E], XTs[ei][:, st * 128:(st + 1) * 128],
                        WV[:, 512 * ei:512 * (ei + 1)],
                        start=(ei == 0), stop=(ei == NEO - 1))
                vsb = blate.tile([128, W65], BF16, tag=f"v{st}", name=f"v{st}")
                nc.vector.tensor_tensor(
                    AP(vsb.tensor, 0, [[W65, 128], [65, H], [1, HD]]),
                    pv[:, 0:E], BVB[:], AT.add)
                nc.gpsimd.memset(
                    AP(vsb.tensor, HD, [[W65, 128], [65, H]]), 1.0)
                VSB[st] = vsb

            def emit_u_tt(b, h, tt, QV, PT, UBP, act_units=(0, 6)):
                """u band for (head, t_tile): one 2-bank psum tile, one copy."""
                hp, i = h // 2, h % 2
                r0 = 64 * i
                j0 = J0[tt]
                lqv = QV[hp][r0:r0 + 64, 128 * tt:128 * (tt + 1)]
                ua = pup.tile([128, 640], F32, tag="pu", name="ua")
                nc.tensor.matmul(ua[:, 0:512], lqv,
                                 PT[hp][r0:r0 + 64, j0:j0 + 512],
                                 start=True, stop=True,
                                 tile_position=(r0, 0))
                nc.tensor.matmul(ua[:, 512:640], lqv,
                                 PT[hp][r0:r0 + 64, j0 + 512:j0 + 640],
                                 start=True, stop=True,
                                 tile_position=(r0, 0))
                base = 1280 * tt + 640 * i
                if ((tt << 1) | i) in act_units:
                    nc.scalar.activation(UBP[:, base:base + 639],
                                         ua[:, 0:639], AF.Copy)
                else:
                    nc.vector.tensor_copy(UBP[:, base:base + 639],
                                          ua[:, 0:639])

            def alloc_ubp(k):
                return ubpp.tile([128, 4 * 1280], BF16, tag="ub",
                                 name=f"ub{k}")

            def emit_shift(k, UBP, split=False):
                """diagonal rel-shift DMA; split per-tt for the startup pairs
                whose shift is on the critical path."""
                vp = vpp.tile([128, 4096], BF16, tag="vp", name=f"vp{k}")
                if split:
                    for tt in range(4):
                        nc.sync.dma_start(
                            out=vp[:, 1024 * tt:1024 * (tt + 1)],
                            in_=AP(UBP.tensor, 127 + 1280 * tt,
                                   [[5119, 128], [640, 2], [1, 512]]))
                else:
                    nc.sync.dma_start(
                        out=vp,
                        in_=AP(UBP.tensor, 127,
                               [[5119, 128], [1280, 4], [640, 2], [1, 512]]))
                return vp

            fillq = []

            def fill():
                if fillq:
                    fillq.pop(0)()

            def flush():
                while fillq:
                    fillq.pop(0)()

            def emit_scores(b, h, VP, QU, KT, pac_pool=None, pac_tag="pac"):
                hp, r0, i = h // 2, 64 * (h % 2), h % 2
                pool = pac_pool if pac_pool is not None else pacs
                PAC = [None] * 4
                ETS = [None] * 4

                def emit_ac(j):
                    pac = pool.tile([128, T], F32, tag=pac_tag, name="pac")
                    nc.tensor.matmul(pac[:],
                                     KT[hp][r0:r0 + 64, 128 * j:128 * (j + 1)],
                                     QU[hp][r0:r0 + 64, :],
                                     start=True, stop=False,
                                     tile_position=(r0, 0),
                                     skip_group_check=True)
                    PAC[j] = pac

                def emit_texp(j):
                    pac = PAC[j]
                    for tt in range(4):
                        nc.tensor.matmul(
                            pac[:, 128 * tt:128 * (tt + 1)],
                            VP[:, 1024 * tt + 512 * i + 128 * j:
                                  1024 * tt + 512 * i + 128 * (j + 1)],
                            IDENT[:],
                            start=False, stop=(tt == 3),
                            skip_group_check=True)
                    ets = epp.tile([128, T], BF16, tag=f"e{j}", name=f"e{h}_{j}")
                    nc.scalar.activation(ets[:], pac[:], AF.Exp,
                                         bias=0.0, scale=SCALE)
                    ETS[j] = ets

                emit_ac(0)
                emit_ac(1)
                emit_texp(0)
                fill()
                emit_ac(2)
                emit_texp(1)
                fill()
                emit_ac(3)
                emit_texp(2)
                fill()
                emit_texp(3)
                fill()
                return ETS

            def emit_av(b, h, ETS, VSB, avc_act=None, avc_bf=False):
                av = pjp.tile([128, 4 * 65], F32, tag="pj", name=f"av{h}")
                for tt in range(4):
                    for j in range(4):
                        nc.tensor.matmul(
                            av[:, 65 * tt:65 * (tt + 1)],
                            ETS[j][:, 128 * tt:128 * (tt + 1)],
                            VSB[j][:, 65 * h:65 * (h + 1)],
                            start=(j == 0), stop=(j == 3))
                    fill()
                avc = work.tile([128, 4 * 65], BF16 if avc_bf else F32,
                                tag="avcb" if avc_bf else "avc",
                                name=f"avc{h}")
                if avc_act is None:
                    avc_act = (h % 2 == 0)
                if avc_act:
                    nc.scalar.activation(avc[:], av[:], AF.Copy)
                else:
                    nc.vector.tensor_copy(avc[:], av[:])
                rec = work.tile([128, 4], F32, tag=f"rec{h}", name=f"rec{h}")
                nc.vector.reciprocal(
                    rec[:], AP(avc.tensor, HD, [[4 * 65, 128], [65, 4]]))
                return avc, rec

            def emit_norm(b, h, avc, rec, OQ, engines=None):
                c0 = 64 * (h % 2)
                for tt in range(4):
                    dst = OQ[:, 128 * tt + c0:128 * tt + c0 + 64]
                    src = avc[:, 65 * tt:65 * tt + 64]
                    if engines and engines[tt] == "dve":
                        nc.vector.tensor_scalar_mul(dst, src, rec[:, tt:tt + 1])
                    else:
                        nc.gpsimd.tensor_scalar_mul(dst, src, rec[:, tt:tt + 1])

            # ================= schedule =================
            # per-batch tile contexts; pair k = (b, hp) = divmod(k, 4)
            ctx = {0: {"QU": [None] * 4, "QV": [None] * 4, "KT": [None] * 4,
                       "PT": [None] * 4, "VSB": [None] * 4,
                       "xs": xs_of(XT0), "ps": ps_of(PET0)}}
            ubp_ring = {}
            vp_ring = {}

            def u_fillers(k):
                """closures for pair k's 8 u_tt groups (order tt-major).
                ACT/DVE copy split tuned per phase load: prologue pairs
                lean on ACT (idle there), late pairs stay 2/8."""
                b, hp = divmod(k, 4)
                ubp_ring[k] = alloc_ubp(k)
                c = ctx[b]
                act_units = (0, 2, 4, 6) if k < 2 else (tuple(u for u in (am0, am1) if u >= 0) if k < 6 else ((am0,) if am1 == 6 else ()))
                res = []
                for tt in range(4):
                    for i in range(2):
                        res.append(lambda tt=tt, i=i, b=b, hp=hp:
                                   emit_u_tt(b, 2 * hp + i, tt,
                                             c["QV"], c["PT"], ubp_ring[k],
                                             act_units))
                return res

            # ---- prologue: batch 0 projections with pair-0/1 u interleave
            # batch-0 p projection, ei-major: each arriving PET/WP chunk is
            # consumed immediately across all four eo tiles (pj ring is 4 deep)
            c0_ = ctx[0]
            for half in range(2):
                c0h = 512 * half
                pph = [(pup if eo < 2 else pjp).tile(
                    [128, 640 if eo < 2 else 512], F32,
                    tag="pu" if eo < 2 else "pj", name=f"pp{eo}")
                       for eo in range(NEO)]
                for ei in range(NEO):
                    for eo in range(NEO):
                        nc.tensor.matmul(
                            pph[eo][:, 0:512],
                            WP[:, 512 * ei + 128 * eo:512 * ei + 128 * (eo + 1)],
                            c0_["ps"][ei][:, c0h:c0h + 512],
                            start=(ei == 0), stop=(ei == NEO - 1),
                            skip_group_check=True)
                for eo in range(NEO):
                    if c0_["PT"][eo] is None:
                        c0_["PT"][eo] = blate.tile([128, S2 + 1], BF16,
                                                   tag=f"pt{eo}",
                                                   name=f"pt0_{eo}")
                    if half == 1:
                        nc.scalar.activation(
                            c0_["PT"][eo][:, c0h:c0h + 512],
                            pph[eo][:, 0:512], AF.Copy)
                    else:
                        nc.vector.tensor_copy(
                            c0_["PT"][eo][:, c0h:c0h + 512], pph[eo][:, 0:512])
            # pair-0 u fillers wait on the qu->qv chain; emit the first q/k
            # groups before draining any so the chain has latency cover.
            emit_q_group(0, c0_["xs"], 0, c0_["QU"], c0_["QV"])
            emit_k_group(0, c0_["xs"], 0, c0_["KT"])
            fillq.extend(u_fillers(0))
            for eo in range(1, NEO):
                emit_q_group(0, c0_["xs"], eo, c0_["QU"], c0_["QV"])
                fill()
                fill()
                emit_k_group(0, c0_["xs"], eo, c0_["KT"])
                fill()
            flush()
            vp_ring[0] = emit_shift(0, ubp_ring[0])
            fillq.extend(u_fillers(1))
            for st in range(NEO):
                emit_v_group(0, c0_["xs"], st, c0_["VSB"])
                fill()
                fill()
            flush()

            def finish_pair(b, hp, ETS0, ETS1, last):
                """av + normalize + output DMA for pair (b, hp) — emitted one
                phase late so the exp chain never gates the av matmuls."""
                cb = ctx[b]
                OQ = osb.tile([128, 512], BF16, tag="oq", name=f"o{b}_{hp}")
                late = 4 * b + hp >= 5
                if not last:
                    avc0, rec0 = emit_av(b, 2 * hp, ETS0, cb["VSB"],
                                         avc_act=False if late else None)
                    avc1, rec1 = emit_av(b, 2 * hp + 1, ETS1, cb["VSB"],
                                         avc_act=False if late else None)
                    emit_norm(b, 2 * hp, avc0, rec0, OQ)
                    emit_norm(b, 2 * hp + 1, avc1, rec1, OQ)
                    nc.sync.dma_start(
                        out=AP(out_d, b * T * E + 128 * hp,
                               [[512, 128], [65536, 4], [1, 128]]),
                        in_=OQ[:])
                else:
                    # tail: per-head epilogue, avc on ACT, norms on DVE,
                    # split per-head DMAs
                    avc0, rec0 = emit_av(b, 2 * hp, ETS0, cb["VSB"],
                                         avc_act=True, avc_bf=True)
                    emit_norm(b, 2 * hp, avc0, rec0, OQ,
                              engines=("dve", "dve", "dve", "dve"))
                    nc.sync.dma_start(
                        out=AP(out_d, b * T * E + 128 * hp,
                               [[512, 128], [65536, 4], [1, 64]]),
                        in_=AP(OQ.tensor, 0, [[512, 128], [128, 4], [1, 64]]))
                    avc1, rec1 = emit_av(b, 2 * hp + 1, ETS1, cb["VSB"],
                                         avc_act=True, avc_bf=True)
                    emit_norm(b, 2 * hp + 1, avc1, rec1, OQ,
                              engines=("dve", "dve", "dve", "dve"))
                    nc.sync.dma_start(
                        out=AP(out_d, b * T * E + 128 * hp + 64,
                               [[512, 128], [65536, 4], [1, 64]]),
                        in_=AP(OQ.tensor, 64, [[512, 128], [128, 4], [1, 64]]))

            # ---- main pair loop (av stage pipelined one phase behind)
            pending_av = None
            sc_cache = {}
            for k in range(NPAIR):
                b, hp = divmod(k, 4)
                last = (k == NPAIR - 1)
                cb = ctx[b]
                if k + 1 < NPAIR:
                    vp_ring[k + 1] = emit_shift(k + 1, ubp_ring[k + 1])
                if hp == 0 and b + 1 < bpc:
                    nb = {"QU": [None] * 4, "QV": [None] * 4,
                          "KT": [None] * 4, "PT": [None] * 4,
                          "VSB": [None] * 4}
                    nb["xs"], nb["ps"] = emit_loads(b + 1)
                    ctx[b + 1] = nb
                # fillers for this phase
                if k + 2 < NPAIR:
                    uf = u_fillers(k + 2)
                else:
                    uf = []
                pf = []
                if b + 1 < bpc:
                    nb = ctx[b + 1]
                    if hp == 1:
                        pf = [
                            (lambda eo=eo, half=half:
                             emit_p_group(b + 1, nb["ps"], eo, half, nb["PT"]))
                            for eo in range(NEO) for half in range(2)]
                    elif hp == 2:
                        for eo in range(NEO):
                            pf.append(lambda eo=eo:
                                      emit_q_group(b + 1, nb["xs"], eo,
                                                   nb["QU"], nb["QV"]))
                            pf.append(lambda eo=eo:
                                      emit_k_group(b + 1, nb["xs"], eo,
                                                   nb["KT"]))
                    elif hp == 3:
                        pf = [(lambda st=st:
                               emit_v_group(b + 1, nb["xs"], st, nb["VSB"]))
                              for st in range(NEO)]
                if hp == 2:
                    fillq.extend(pf)      # qk must precede next batch's u
                    fillq.extend(uf)
                else:
                    fillq.extend(uf)
                    fillq.extend(pf)

                if k in sc_cache:
                    ETS0, ETS1 = sc_cache.pop(k)
                else:
                    VP = vp_ring.pop(k)
                    ETS0 = emit_scores(b, 2 * hp, VP, cb["QU"], cb["KT"])
                    ETS1 = emit_scores(b, 2 * hp + 1, VP, cb["QU"], cb["KT"])
                flush()
                if k == NPAIR - 2:
                    pass
                if pending_av is not None and k != NPAIR - 2:
                    finish_pair(*pending_av, last=False)
                if k == NPAIR - 2:
                    # pre-emit the last pair's scores on the idle pup ring so
                    # its exp chain overlaps this phase instead of walling the
                    # endgame on ACT.
                    b2, hp2 = divmod(k + 1, 4)
                    c2 = ctx[b2]
                    VP2 = vp_ring.pop(k + 1)
                    E0 = emit_scores(b2, 2 * hp2, VP2, c2["QU"], c2["KT"],
                                     pac_pool=pup, pac_tag="pu")
                    E1 = emit_scores(b2, 2 * hp2 + 1, VP2, c2["QU"], c2["KT"],
                                     pac_pool=pup, pac_tag="pu")
                    sc_cache[k + 1] = (E0, E1)
                    finish_pair(*pending_av, last=False)
                pending_av = (b, hp, ETS0, ETS1)
                flush()
            finish_pair(*pending_av, last=True)

    _split_multiwaits(nc, mybir)
    return nc


def _prep_inputs(x, pos_emb, Wq, bq, Wk, bk, Wv, bv, Wp,
                 pos_bias_u, pos_bias_v):
    import ml_dtypes
    BF = ml_dtypes.bfloat16
    xT = np.ascontiguousarray(
        np.asarray(x, np.float32).transpose(0, 2, 1)).astype(BF)
    peT = np.zeros((B, E, S2 + 1), BF)
    peT[:, :, 0:S2] = np.asarray(pos_emb, np.float32).transpose(0, 2, 1).astype(BF)
    wqT = np.ascontiguousarray(np.asarray(Wq, np.float32).T.astype(BF))
    wkT = np.ascontiguousarray(np.asarray(Wk, np.float32).T.astype(BF))
    wvT = np.ascontiguousarray(np.asarray(Wv, np.float32).T.astype(BF))
    wpT = np.ascontiguousarray(np.asarray(Wp, np.float32).T.astype(BF))
    bias_u = (np.asarray(bq, np.float32)
              + np.asarray(pos_bias_u, np.float32).reshape(E))
    bias_v = (np.asarray(bq, np.float32)
              + np.asarray(pos_bias_v, np.float32).reshape(E))
    bu_p = np.ascontiguousarray(bias_u.reshape(4, 128).T)
    bdv_p = np.ascontiguousarray((bias_v - bias_u).reshape(4, 128).T)
    bk_p = np.ascontiguousarray(np.asarray(bk, np.float32).reshape(4, 128).T)
    common = {
        "wqT": wqT, "wkT": wkT, "wvT": wvT, "wpT": wpT,
        "bu": bu_p, "bdv": bdv_p, "bkk": bk_p,
        "bvp": np.ascontiguousarray(bias_v.reshape(4, 128).T),
        "bvec": np.asarray(bv, np.float32),
    }
    in_maps = []
    for c in range(N_CORES):
        m = dict(common)
        m["xT"] = xT[c * BPC:(c + 1) * BPC]
        m["peT"] = peT[c * BPC:(c + 1) * BPC]
        in_maps.append(m)
    return in_maps


def kernel(x, pos_emb, Wq, bq, Wk, bk, Wv, bv, Wp,
           pos_bias_u, pos_bias_v, legacy=0, **_):
    from concourse.bass_utils import run_bass_kernel_spmd

    if "nc" not in _CACHE:
        _CACHE["nc"] = _build_nc()
    nc = _CACHE["nc"]
    in_maps = _prep_inputs(x, pos_emb, Wq, bq, Wk, bk, Wv, bv, Wp,
                           pos_bias_u, pos_bias_v)
    res = run_bass_kernel_spmd(nc, in_maps, list(range(N_CORES))).results
    return np.concatenate(
        [np.asarray(r["out"], np.float32) for r in res], axis=0)   # [B, T, E]
